# revision 20
# baseline (speedup 1.0000x reference)
"""Multi-head attention (B=2, S=2048, H=1024, 16 heads) on 8 TRN2 NeuronCores.

Sharding: data-parallel over batch (2) x tensor-parallel over heads (16 -> 4
groups of 4 heads).  Core c = b*4 + g handles batch b, heads [4g, 4g+4).

v2 structure (fp16 storage / fp32 accumulate), x = q|k|v of its batch:
  QP_T[d, s] = (Wq_g x^T + bq_g)   d on partitions
  KP_T[d, s] = (Wk_g x^T + bk_g)
  VP[s, d]   = x Wv_g^T            natural layout + ones column per head
  per (pair p, i-block, j-tile):
     S_T[j, i] = K^T-contracted scores  (two heads share the PE via row tiles)
     E = exp(S_T/8)  on ACT, fused over two j-tiles  ([128, 2048] per instr)
     O[q, d+1] += E_j^T @ [V_j | 1]    transposed AV: q on psum partitions,
                                       ones column accumulates the denom L
  norm: r = 1/L (custom-DVE fast reciprocal), O_n = O * r (tensor_scalar),
        O_n^T via DMA-transpose (xbar) -> onorm[d, q]
  out_T[o, i] = Wo_g^T-contracted projection of onorm -> [1024, 2048] f32->f16
Host: out[b] = sum_g out_T(b,g)^T + (Wo @ bv + bo).
"""

import json

import numpy as np

S = 2048
H = 1024
DL = 256          # local projection dim = 4 heads * 64
P = 128
HD = 64
NK = H // P       # 8 k-tiles over hidden dim
NB = 512          # i-block (queries per attention unit column block)
NJ = S // P       # 16 j tiles of 128 keys
VW = 4 * (HD + 1)  # vp row width: 4 heads x (64 + ones col)
RS = 72           # AV psum region stride (32B aligned); region width 65

_nc_cache = {}
VARIANT = "full"  # debug bisect hook


# --------------------------------------------------------------------------
# BIR fix: this container's walrus supports only ONE sync wait (and update)
# per TPB instruction; Tile attaches several.  Split extras onto single-wait
# EventSemaphore instructions at the serialization boundary.
# --------------------------------------------------------------------------
_wsplit_counter = [0]


def _mk_evsem(engine, debug, wait=None, update=None):
    _wsplit_counter[0] += 1
    return {
        "debug": debug,
        "engine": engine,
        "ins": [],
        "outs": [],
        "name": f"wsplit-{_wsplit_counter[0]}",
        "opcode": "EventSemaphore",
        "sync_info": {
            "on_wait": [wait] if wait else [],
            "on_update": [update] if update else [],
        },
    }


def _split_bir_waits(bir):
    for f in bir.get("functions", []):
        for blk in f.get("blocks", []):
            out = []
            for inst in blk.get("instructions", []):
                si = inst.get("sync_info")
                waits = list(si.get("on_wait") or []) if si else []
                updates = list(si.get("on_update") or []) if si else []
                eng = inst.get("engine")
                dbg = inst.get("debug", 0)
                if len(waits) > 1:
                    for w in waits[:-1]:
                        out.append(_mk_evsem(eng, dbg, wait=w))
                    si["on_wait"] = [waits[-1]]
                out.append(inst)
                if len(updates) > 1:
                    si["on_update"] = [updates[0]]
                    for u in updates[1:]:
                        out.append(_mk_evsem(eng, dbg, update=u))
            blk["instructions"] = out
    return bir


def _install_bir_fix():
    import concourse.bass as bass

    if getattr(bass.Bass, "_wsplit_installed", False):
        return
    orig = bass.Bass.to_json_bytes

    def to_json_bytes(self, *a, **k):
        bir = json.loads(orig(self, *a, **k))
        return json.dumps(_split_bir_waits(bir)).encode()

    bass.Bass.to_json_bytes = to_json_bytes
    bass.Bass._wsplit_installed = True


# --------------------------------------------------------------------------
# Kernel builder
# --------------------------------------------------------------------------

def _build_nc():
    import concourse.bass as bass
    import concourse.mybir as mybir
    import concourse.tile as tile

    f16 = mybir.dt.float16
    f32 = mybir.dt.float32
    Exp = mybir.ActivationFunctionType.Exp

    nc = bass.Bass("TRN2")

    xq = nc.dram_tensor("xq", [H, S], f16, kind="ExternalInput")
    xk = nc.dram_tensor("xk", [H, S], f16, kind="ExternalInput")
    xv = nc.dram_tensor("xv", [H, S], f16, kind="ExternalInput")
    wqT = nc.dram_tensor("wqT", [H, DL], f16, kind="ExternalInput")
    wkT = nc.dram_tensor("wkT", [H, DL], f16, kind="ExternalInput")
    wvT = nc.dram_tensor("wvT", [H, DL], f16, kind="ExternalInput")
    woT = nc.dram_tensor("woT", [DL, H], f16, kind="ExternalInput")
    bias = nc.dram_tensor("bias", [P, 4], f32, kind="ExternalInput")  # bq0 bq1 bk0 bk1
    out = nc.dram_tensor("out", [H, S], f16, kind="ExternalOutput")
    dbg = {}
    if VARIANT == "debug":
        dbg["qpt"] = nc.dram_tensor("dbg_qpt", [P, 2 * S], f16, kind="ExternalOutput")
        dbg["kpt"] = nc.dram_tensor("dbg_kpt", [P, 2 * S], f16, kind="ExternalOutput")
        dbg["vp"] = nc.dram_tensor("dbg_vp", [P, NJ * VW], f16, kind="ExternalOutput")
        dbg["e0"] = nc.dram_tensor("dbg_e0", [P, 1024], f16, kind="ExternalOutput")
        dbg["onq0"] = nc.dram_tensor("dbg_onq0", [P, 4 * P], f16, kind="ExternalOutput")
        dbg["onorm"] = nc.dram_tensor("dbg_onorm", [P, 2 * S], f16, kind="ExternalOutput")
        dbg["r0"] = nc.dram_tensor("dbg_r0", [P, 8], f32, kind="ExternalOutput")
        dbg["avp0"] = nc.dram_tensor("dbg_avp0", [P, 2 * 512], f32, kind="ExternalOutput")

    UNITS = [(p, ib) for p in range(2) for ib in range(4)]  # p-major

    with tile.TileContext(nc) as tc:
        with (
            tc.tile_pool(name="persist", bufs=1) as persist,
            tc.tile_pool(name="xpool", bufs=96) as xpool,
            tc.tile_pool(name="epool", bufs=6) as epool,
            tc.tile_pool(name="onq", bufs=2) as onqpool,
            tc.tile_pool(name="rp", bufs=2) as rpool,
            tc.tile_pool(name="otp", bufs=3) as otpool,
            tc.tile_pool(name="scp", bufs=1, space="PSUM") as scpool,
            tc.tile_pool(name="avp", bufs=1, space="PSUM") as avpool,
            tc.tile_pool(name="opsp", bufs=1, space="PSUM") as opspool,
        ):
            # ---- persistent tiles ----
            wq_sb = persist.tile([P, NK, DL], f16, name="wq_sb")
            wk_sb = persist.tile([P, NK, DL], f16, name="wk_sb")
            wv_sb = persist.tile([P, NK, DL], f16, name="wv_sb")
            wo_sb = persist.tile([P, 2, H], f16, name="wo_sb")
            bias_sb = persist.tile([P, 4], f32, name="bias_sb")
            qpt = persist.tile([P, 2, S], f16, name="qpt")
            kpt = persist.tile([P, 2, S], f16, name="kpt")
            vp = persist.tile([P, NJ, VW], f16, name="vp")
            onorm = persist.tile([P, 2, S], f16, name="onorm")

            # psum: SC double-buffer (2x2 banks) + AV ping-pong (3 banks)
            SC = scpool.tile([P, 2, 1024], f32, name="SC")
            AVP = avpool.tile([P, 3, 512], f32, name="AVP")

            # ---- DMA loads, first-needed first ----
            nc.sync.dma_start(wv_sb[:], wvT.rearrange("(k p) d -> p k d", p=P))
            xv_q = {}
            xk_q = {}
            xq_q = {}

            def load_q(dst, src, k, n):
                t = xpool.tile([P, NB], f16, name="x_sb")
                nc.sync.dma_start(t[:], src[k * P:(k + 1) * P, n * NB:(n + 1) * NB])
                dst[(k, n)] = t

            for k in range(NK):
                load_q(xv_q, xv, k, 0)
            nc.sync.dma_start(wk_sb[:], wkT.rearrange("(k p) d -> p k d", p=P))
            for k in range(NK):
                load_q(xk_q, xk, k, 0)
            nc.sync.dma_start(wq_sb[:], wqT.rearrange("(k p) d -> p k d", p=P))
            for k in range(NK):
                load_q(xq_q, xq, k, 0)
            nc.sync.dma_start(bias_sb[:], bias[:])
            for n in range(1, 4):
                for k in range(NK):
                    load_q(xk_q, xk, k, n)
                for k in range(NK):
                    load_q(xv_q, xv, k, n)
                for k in range(NK):
                    load_q(xq_q, xq, k, n)
            nc.sync.dma_start(wo_sb[:], woT.rearrange("(k p) d -> p k d", p=P))

            # ones columns in vp (col 64 of each head block)
            for h in range(4):
                nc.gpsimd.memset(vp[:, :, h * (HD + 1) + HD:h * (HD + 1) + HD + 1], 1.0)

            # ---- AV psum region map: parity ping-pong over 3 banks ----
            def av_reg(par, t):
                if par == 0:
                    if t < 7:
                        return AVP[:, 0, RS * t:RS * t + 65]
                    return AVP[:, 1, 0:65]
                else:
                    if t < 6:
                        return AVP[:, 1, RS + RS * t:RS + RS * t + 65]
                    return AVP[:, 2, RS * (t - 6):RS * (t - 6) + 65]

            # ---- V projection for one s-tile ----
            def v_block(s):
                n, c = divmod(s, 4)
                ps = opspool.tile([P, NB], f32, name="ops_t")
                for k in range(NK):
                    nc.tensor.matmul(
                        ps[:, :DL],
                        xv_q[(k, n)][:, c * P:(c + 1) * P],
                        wv_sb[:, k, :],
                        start=(k == 0),
                        stop=(k == NK - 1),
                    )
                for hg in range(4):
                    nc.vector.tensor_copy(
                        vp[:, s, hg * (HD + 1):hg * (HD + 1) + HD],
                        ps[:, hg * HD:(hg + 1) * HD],
                    )

            # ---- K/Q projection, one (m, n) block ----
            def proj_qk_n(wsb, xq_, dst, bcol, m, n):
                ps = opspool.tile([P, NB], f32, name="ops_t")
                for k in range(NK):
                    nc.tensor.matmul(
                        ps[:],
                        wsb[:, k, m * P:(m + 1) * P],
                        xq_[(k, n)][:],
                        start=(k == 0),
                        stop=(k == NK - 1),
                    )
                nc.vector.tensor_scalar_add(
                    dst[:, m, n * NB:(n + 1) * NB], ps[:],
                    bias_sb[:, bcol + m:bcol + m + 1],
                )

            # ---- attention pieces ----
            e_tiles = {}

            def scores_j(u, j):
                p, ib = UNITS[u]
                slot = (u * NJ + j) % 2
                isl = slice(ib * NB, (ib + 1) * NB)
                jsl = slice(j * P, (j + 1) * P)
                nc.tensor.matmul(
                    SC[:, slot, 0:NB],
                    kpt[0:HD, p, jsl], qpt[0:HD, p, isl],
                    start=True, stop=True,
                )
                nc.tensor.matmul(
                    SC[:, slot, NB:2 * NB],
                    kpt[HD:P, p, jsl], qpt[HD:P, p, isl],
                    start=True, stop=True,
                )

            def exp_j(u, j):
                slot = (u * NJ + j) % 2
                e = epool.tile([P, 1024], f16, name="e_t")
                nc.scalar.activation(e[:], SC[:, slot, :], Exp, scale=0.125)
                e_tiles[(u, j)] = e
                if VARIANT == "debug" and (u, j) == (0, 0):
                    nc.sync.dma_start(dbg["e0"][:], e[:])

            def av_j(u, j):
                p, ib = UNITS[u]
                par = u % 2
                e = e_tiles.pop((u, j))
                for t in range(8):
                    qs, hl = divmod(t, 2)
                    lhsT = e[:, hl * NB + qs * P:hl * NB + (qs + 1) * P]
                    hg = 2 * p + hl
                    # start=True would zero the WHOLE psum bank, wiping the
                    # sibling regions packed in it — accumulate over memset.
                    nc.tensor.matmul(
                        av_reg(par, t),
                        lhsT,
                        vp[:, j, hg * (HD + 1):hg * (HD + 1) + 65],
                        start=False,
                        stop=(j == NJ - 1),
                        skip_group_check=True,
                    )

            def av_zero(u):
                if u % 2 == 0:
                    nc.vector.memset(AVP[:, 0, 0:6 * RS + 65], 0.0)
                    nc.vector.memset(AVP[:, 1, 0:65], 0.0)
                else:
                    nc.vector.memset(AVP[:, 1, RS:6 * RS + 65], 0.0)
                    nc.vector.memset(AVP[:, 2, 0:RS + 65], 0.0)

            def flat(ap):
                return ap.rearrange("p t c -> p (t c)")

            def _recip(out_ap, in_ap):
                # custom-DVE (reciprocal_approx_fast) hits "ISA wrong length"
                # in this container's walrus; plain InstReciprocal on [128,<=8]
                # is cheap enough.
                nc.vector.reciprocal(out_ap, in_ap)

            def _otrans(dst, src):
                if VARIANT == "notrans":
                    nc.sync.dma_start(dst, src.rearrange("a b -> b a"))
                else:
                    nc.sync.dma_start_transpose(dst, src)

            def norm(u):
                p, ib = UNITS[u]
                par = u % 2
                r = rpool.tile([P, 8], f32, name="r_t")
                if par == 0:
                    l0 = AVP[:, 0, 0:7 * RS].rearrange(
                        "p (t c) -> p t c", c=RS)[:, :, 64:65]
                    _recip(r[:, 0:7], flat(l0))
                    _recip(r[:, 7:8], AVP[:, 1, 64:65])
                else:
                    l0 = AVP[:, 1, RS:7 * RS].rearrange(
                        "p (t c) -> p t c", c=RS)[:, :, 64:65]
                    _recip(r[:, 0:6], flat(l0))
                    l1 = AVP[:, 2, 0:2 * RS].rearrange(
                        "p (t c) -> p t c", c=RS)[:, :, 64:65]
                    _recip(r[:, 6:8], flat(l1))
                o_nq = onqpool.tile([P, 4, P], f16, name="onq_t")
                for t in range(8):
                    qs, hl = divmod(t, 2)
                    nc.vector.tensor_scalar_mul(
                        o_nq[:, qs, hl * HD:(hl + 1) * HD],
                        av_reg(par, t)[:, 0:HD],
                        r[:, t:t + 1],
                    )
                if VARIANT == "debug" and u == 0:
                    avdump = onqpool.tile([P, 2, 512], f32, name="avdump_t")
                    nc.vector.tensor_copy(avdump[:], AVP[:, 0:2, :])
                    nc.sync.dma_start(dbg["avp0"][:], avdump[:].rearrange("p a b -> p (a b)"))
                    nc.sync.dma_start(dbg["onq0"][:], o_nq[:].rearrange("p a b -> p (a b)"))
                    nc.sync.dma_start(dbg["r0"][:], r[:])
                for qs in range(4):
                    _otrans(
                        onorm[:, p, ib * NB + qs * P:ib * NB + (qs + 1) * P],
                        o_nq[:, qs, :],
                    )

            # ---- Wo: one mo sub-block of output block n ----
            def wo_mm(n, mo):
                ps = opspool.tile([P, NB], f32, name="ops_t")
                for k2 in range(2):
                    nc.tensor.matmul(
                        ps[:],
                        wo_sb[:, k2, mo * P:(mo + 1) * P],
                        onorm[:, k2, n * NB:(n + 1) * NB],
                        start=(k2 == 0),
                        stop=(k2 == 1),
                    )
                ot = otpool.tile([P, NB], f16, name="ot_t")
                nc.vector.tensor_copy(ot[:], ps[:])
                nc.sync.dma_start(out[mo * P:(mo + 1) * P, n * NB:(n + 1) * NB], ot[:])

            # ---- schedule ----
            def P_(wsb, xd, dst, bcol, m, n):
                return lambda: proj_qk_n(wsb, xd, dst, bcol, m, n)

            def V_(s):
                return lambda: v_block(s)

            def W_(n, mo):
                return lambda: wo_mm(n, mo)

            K = lambda m, n: P_(wk_sb, xk_q, kpt, 2, m, n)
            Q = lambda m, n: P_(wq_sb, xq_q, qpt, 0, m, n)

            # fills keyed by global j-step gj = u*16 + j  (0..127).
            # HARD RULE: the tile framework does NOT reorder; every producer
            # must be EMITTED before its consumer.  v_block(s) before
            # av_j(0, s) at gj=s+1; K(p, n) before scores_j((p,*), 4n);
            # Q(p, ib) before gj=unit*16.
            fills = {
                0: [V_(4)],
                1: [V_(5), K(0, 1)],
                2: [V_(6)],
                3: [V_(7)],
                4: [V_(8), K(0, 2)],
                5: [V_(9)],
                6: [V_(10)],
                7: [V_(11), K(0, 3)],
                8: [V_(12)],
                9: [V_(13)],
                10: [V_(14)],
                11: [V_(15)],
                13: [Q(0, 1)],
                28: [Q(0, 2)],
                34: [K(1, 0)],
                38: [K(1, 1)],
                42: [Q(0, 3)],
                46: [K(1, 2)],
                50: [K(1, 3)],
                54: [Q(1, 0)],
                58: [Q(1, 1)],
                62: [Q(1, 2)],
                66: [Q(1, 3)],
            }
            # Wo blocks n=0..2 spread into units 5..7 (after norm of (1, n));
            # wo(n) ready after norm(4+n) which is emitted at gj=(5+n)*16.
            for n in range(3):
                base = (5 + n) * 16 + 1
                for mo in range(8):
                    fills.setdefault(base + mo * 15 // 8, []).append(W_(n, mo))

            # warmup
            for s in range(4):
                v_block(s)
            K(0, 0)()
            Q(0, 0)()

            for gj in range(128):
                u, j = divmod(gj, NJ)
                if j == 0:
                    av_zero(u)
                scores_j(u, j)
                exp_j(u, j)
                for f in fills.get(gj, []):
                    f()
                if gj >= 1:
                    u2, j2 = divmod(gj - 1, NJ)
                    av_j(u2, j2)
                    if j2 == NJ - 1:
                        norm(u2)

            # tail
            av_j(7, NJ - 1)
            norm(7)
            for mo in range(8):
                wo_mm(3, mo)
            if VARIANT == "debug":
                nc.sync.dma_start(dbg["qpt"][:], qpt[:].rearrange("p a b -> p (a b)"))
                nc.sync.dma_start(dbg["kpt"][:], kpt[:].rearrange("p a b -> p (a b)"))
                nc.sync.dma_start(dbg["vp"][:], vp[:].rearrange("p a b -> p (a b)"))
                nc.sync.dma_start(dbg["onorm"][:], onorm[:].rearrange("p a b -> p (a b)"))

    return nc


def _get_nc():
    if "nc" not in _nc_cache:
        _install_bir_fix()
        _nc_cache["nc"] = _build_nc()
    return _nc_cache["nc"]


# --------------------------------------------------------------------------
# Host wrapper
# --------------------------------------------------------------------------
def run(inputs, trace=False):
    from concourse.bass_utils import run_bass_kernel_spmd

    q = np.asarray(inputs["q"], np.float32)
    k = np.asarray(inputs["k"], np.float32)
    v = np.asarray(inputs["v"], np.float32)
    Wq = np.asarray(inputs["Wq"], np.float32)
    bq = np.asarray(inputs["bq"], np.float32)
    Wk = np.asarray(inputs["Wk"], np.float32)
    bk = np.asarray(inputs["bk"], np.float32)
    Wv = np.asarray(inputs["Wv"], np.float32)
    bv = np.asarray(inputs["bv"], np.float32)
    Wo = np.asarray(inputs["Wo"], np.float32)
    bo = np.asarray(inputs["bo"], np.float32)

    nc = _get_nc()

    xT = {}
    for b in range(2):
        xT[b] = (
            np.ascontiguousarray(q[b].T).astype(np.float16),
            np.ascontiguousarray(k[b].T).astype(np.float16),
            np.ascontiguousarray(v[b].T).astype(np.float16),
        )

    in_maps = []
    for c in range(8):
        b, g = divmod(c, 4)
        sl = slice(g * DL, (g + 1) * DL)
        bias = np.stack(
            [bq[sl][:P], bq[sl][P:], bk[sl][:P], bk[sl][P:]], axis=1
        ).astype(np.float32)
        in_maps.append({
            "xq": xT[b][0],
            "xk": xT[b][1],
            "xv": xT[b][2],
            "wqT": np.ascontiguousarray(Wq[sl, :].T).astype(np.float16),
            "wkT": np.ascontiguousarray(Wk[sl, :].T).astype(np.float16),
            "wvT": np.ascontiguousarray(Wv[sl, :].T).astype(np.float16),
            "woT": np.ascontiguousarray(Wo[:, sl].T).astype(np.float16),
            "bias": bias,
        })

    res = run_bass_kernel_spmd(
        nc, in_maps, core_ids=list(range(8)), trace=trace,
    )
    outs = [r["out"] for r in res.results]

    const = (Wo @ bv + bo).astype(np.float32)  # [1024]
    full = np.empty((2, S, H), np.float32)
    for b in range(2):
        acc = outs[4 * b].astype(np.float32).copy()
        for g in range(1, 4):
            acc += outs[4 * b + g]
        full[b] = acc.T + const
    return full, res


def kernel(**inputs):
    full, _ = run(inputs, trace=False)
    return full


# revision 22
# speedup vs baseline: 1.0043x; 1.0043x over previous
"""Multi-head attention (B=2, S=2048, H=1024, 16 heads) on 8 TRN2 NeuronCores.

Sharding: data-parallel over batch (2) x tensor-parallel over heads (16 -> 4
groups of 4 heads).  Core c = b*4 + g handles batch b, heads [4g, 4g+4).

v2 structure (fp16 storage / fp32 accumulate), x = q|k|v of its batch:
  QP_T[d, s] = (Wq_g x^T + bq_g)   d on partitions
  KP_T[d, s] = (Wk_g x^T + bk_g)
  VP[s, d]   = x Wv_g^T            natural layout + ones column per head
  per (pair p, i-block, j-tile):
     S_T[j, i] = K^T-contracted scores  (two heads share the PE via row tiles)
     E = exp(S_T/8)  on ACT, fused over two j-tiles  ([128, 2048] per instr)
     O[q, d+1] += E_j^T @ [V_j | 1]    transposed AV: q on psum partitions,
                                       ones column accumulates the denom L
  norm: r = 1/L (custom-DVE fast reciprocal), O_n = O * r (tensor_scalar),
        O_n^T via DMA-transpose (xbar) -> onorm[d, q]
  out_T[o, i] = Wo_g^T-contracted projection of onorm -> [1024, 2048] f32->f16
Host: out[b] = sum_g out_T(b,g)^T + (Wo @ bv + bo).
"""

import json

import numpy as np

S = 2048
H = 1024
DL = 256          # local projection dim = 4 heads * 64
P = 128
HD = 64
NK = H // P       # 8 k-tiles over hidden dim
NB = 512          # i-block (queries per attention unit column block)
NJ = S // P       # 16 j tiles of 128 keys
VW = 4 * (HD + 1)  # vp row width: 4 heads x (64 + ones col)
RS = 72           # AV psum region stride (32B aligned); region width 65

_nc_cache = {}
VARIANT = "full"  # debug bisect hook


# --------------------------------------------------------------------------
# BIR fix: this container's walrus supports only ONE sync wait (and update)
# per TPB instruction; Tile attaches several.  Split extras onto single-wait
# EventSemaphore instructions at the serialization boundary.
# --------------------------------------------------------------------------
_wsplit_counter = [0]


def _mk_evsem(engine, debug, wait=None, update=None):
    _wsplit_counter[0] += 1
    return {
        "debug": debug,
        "engine": engine,
        "ins": [],
        "outs": [],
        "name": f"wsplit-{_wsplit_counter[0]}",
        "opcode": "EventSemaphore",
        "sync_info": {
            "on_wait": [wait] if wait else [],
            "on_update": [update] if update else [],
        },
    }


def _split_bir_waits(bir):
    for f in bir.get("functions", []):
        for blk in f.get("blocks", []):
            out = []
            for inst in blk.get("instructions", []):
                si = inst.get("sync_info")
                waits = list(si.get("on_wait") or []) if si else []
                updates = list(si.get("on_update") or []) if si else []
                eng = inst.get("engine")
                dbg = inst.get("debug", 0)
                if len(waits) > 1:
                    for w in waits[:-1]:
                        out.append(_mk_evsem(eng, dbg, wait=w))
                    si["on_wait"] = [waits[-1]]
                out.append(inst)
                if len(updates) > 1:
                    si["on_update"] = [updates[0]]
                    for u in updates[1:]:
                        out.append(_mk_evsem(eng, dbg, update=u))
            blk["instructions"] = out
    return bir


def _install_bir_fix():
    import concourse.bass as bass

    if getattr(bass.Bass, "_wsplit_installed", False):
        return
    orig = bass.Bass.to_json_bytes

    def to_json_bytes(self, *a, **k):
        bir = json.loads(orig(self, *a, **k))
        return json.dumps(_split_bir_waits(bir)).encode()

    bass.Bass.to_json_bytes = to_json_bytes
    bass.Bass._wsplit_installed = True


# --------------------------------------------------------------------------
# Kernel builder
# --------------------------------------------------------------------------

def _build_nc():
    import concourse.bass as bass
    import concourse.mybir as mybir
    import concourse.tile as tile

    f16 = mybir.dt.float16
    f32 = mybir.dt.float32
    Exp = mybir.ActivationFunctionType.Exp

    nc = bass.Bass("TRN2")

    xq = nc.dram_tensor("xq", [H, S], f16, kind="ExternalInput")
    xk = nc.dram_tensor("xk", [H, S], f16, kind="ExternalInput")
    xv = nc.dram_tensor("xv", [H, S], f16, kind="ExternalInput")
    wqT = nc.dram_tensor("wqT", [H, DL], f16, kind="ExternalInput")
    wkT = nc.dram_tensor("wkT", [H, DL], f16, kind="ExternalInput")
    wvT = nc.dram_tensor("wvT", [H, DL], f16, kind="ExternalInput")
    woT = nc.dram_tensor("woT", [DL, H], f16, kind="ExternalInput")
    bias = nc.dram_tensor("bias", [P, 4], f32, kind="ExternalInput")  # bq0 bq1 bk0 bk1
    out = nc.dram_tensor("out", [H, S], f16, kind="ExternalOutput")
    dbg = {}
    if VARIANT == "debug":
        dbg["qpt"] = nc.dram_tensor("dbg_qpt", [P, 2 * S], f16, kind="ExternalOutput")
        dbg["kpt"] = nc.dram_tensor("dbg_kpt", [P, 2 * S], f16, kind="ExternalOutput")
        dbg["vp"] = nc.dram_tensor("dbg_vp", [P, NJ * VW], f16, kind="ExternalOutput")
        dbg["e0"] = nc.dram_tensor("dbg_e0", [P, 1024], f16, kind="ExternalOutput")
        dbg["onq0"] = nc.dram_tensor("dbg_onq0", [P, 4 * P], f16, kind="ExternalOutput")
        dbg["onorm"] = nc.dram_tensor("dbg_onorm", [P, 2 * S], f16, kind="ExternalOutput")
        dbg["r0"] = nc.dram_tensor("dbg_r0", [P, 8], f32, kind="ExternalOutput")
        dbg["avp0"] = nc.dram_tensor("dbg_avp0", [P, 2 * 512], f32, kind="ExternalOutput")

    UNITS = [(p, ib) for p in range(2) for ib in range(4)]  # p-major

    with tile.TileContext(nc) as tc:
        with (
            tc.tile_pool(name="persist", bufs=1) as persist,
            tc.tile_pool(name="xpool", bufs=96) as xpool,
            tc.tile_pool(name="epool", bufs=6) as epool,
            tc.tile_pool(name="onq", bufs=2) as onqpool,
            tc.tile_pool(name="rp", bufs=2) as rpool,
            tc.tile_pool(name="otp", bufs=3) as otpool,
            tc.tile_pool(name="scp", bufs=1, space="PSUM") as scpool,
            tc.tile_pool(name="avp", bufs=1, space="PSUM") as avpool,
            tc.tile_pool(name="opsp", bufs=1, space="PSUM") as opspool,
        ):
            # ---- persistent tiles ----
            wq_sb = persist.tile([P, NK, DL], f16, name="wq_sb")
            wk_sb = persist.tile([P, NK, DL], f16, name="wk_sb")
            wv_sb = persist.tile([P, NK, DL], f16, name="wv_sb")
            wo_sb = persist.tile([P, 2, H], f16, name="wo_sb")
            bias_sb = persist.tile([P, 4], f32, name="bias_sb")
            qpt = persist.tile([P, 2, S], f16, name="qpt")
            kpt = persist.tile([P, 2, S], f16, name="kpt")
            vp = persist.tile([P, NJ, VW], f16, name="vp")
            onorm = persist.tile([P, 2, S], f16, name="onorm")

            # psum: SC double-buffer (2x2 banks) + AV ping-pong (3 banks)
            SC = scpool.tile([P, 2, 1024], f32, name="SC")
            AVP = avpool.tile([P, 3, 512], f32, name="AVP")

            # ---- DMA loads, first-needed first ----
            nc.sync.dma_start(wv_sb[:], wvT.rearrange("(k p) d -> p k d", p=P))
            xv_q = {}
            xk_q = {}
            xq_q = {}

            def load_q(dst, src, k, n):
                t = xpool.tile([P, NB], f16, name="x_sb")
                nc.sync.dma_start(t[:], src[k * P:(k + 1) * P, n * NB:(n + 1) * NB])
                dst[(k, n)] = t

            for k in range(NK):
                load_q(xv_q, xv, k, 0)
            nc.sync.dma_start(wk_sb[:], wkT.rearrange("(k p) d -> p k d", p=P))
            for k in range(NK):
                load_q(xk_q, xk, k, 0)
            nc.sync.dma_start(wq_sb[:], wqT.rearrange("(k p) d -> p k d", p=P))
            for k in range(NK):
                load_q(xq_q, xq, k, 0)
            nc.sync.dma_start(bias_sb[:], bias[:])
            for n in range(1, 4):
                for k in range(NK):
                    load_q(xk_q, xk, k, n)
                for k in range(NK):
                    load_q(xv_q, xv, k, n)
                for k in range(NK):
                    load_q(xq_q, xq, k, n)
            nc.sync.dma_start(wo_sb[:], woT.rearrange("(k p) d -> p k d", p=P))

            # ones columns in vp (col 64 of each head block)
            for h in range(4):
                nc.gpsimd.memset(vp[:, :, h * (HD + 1) + HD:h * (HD + 1) + HD + 1], 1.0)

            # ---- AV psum region map: parity ping-pong over 3 banks ----
            def av_reg(par, t):
                if par == 0:
                    if t < 7:
                        return AVP[:, 0, RS * t:RS * t + 65]
                    return AVP[:, 1, 0:65]
                else:
                    if t < 6:
                        return AVP[:, 1, RS + RS * t:RS + RS * t + 65]
                    return AVP[:, 2, RS * (t - 6):RS * (t - 6) + 65]

            # ---- V projection for one s-tile ----
            def v_block(s):
                n, c = divmod(s, 4)
                ps = opspool.tile([P, NB], f32, name="ops_t")
                for k in range(NK):
                    nc.tensor.matmul(
                        ps[:, :DL],
                        xv_q[(k, n)][:, c * P:(c + 1) * P],
                        wv_sb[:, k, :],
                        start=(k == 0),
                        stop=(k == NK - 1),
                    )
                for hg in range(4):
                    nc.vector.tensor_copy(
                        vp[:, s, hg * (HD + 1):hg * (HD + 1) + HD],
                        ps[:, hg * HD:(hg + 1) * HD],
                    )

            # ---- K/Q projection, one (m, n) block ----
            def proj_qk_n(wsb, xq_, dst, bcol, m, n):
                ps = opspool.tile([P, NB], f32, name="ops_t")
                for k in range(NK):
                    nc.tensor.matmul(
                        ps[:],
                        wsb[:, k, m * P:(m + 1) * P],
                        xq_[(k, n)][:],
                        start=(k == 0),
                        stop=(k == NK - 1),
                    )
                nc.vector.tensor_scalar_add(
                    dst[:, m, n * NB:(n + 1) * NB], ps[:],
                    bias_sb[:, bcol + m:bcol + m + 1],
                )

            # ---- attention pieces ----
            e_tiles = {}

            def scores_j(u, j):
                p, ib = UNITS[u]
                slot = (u * NJ + j) % 2
                isl = slice(ib * NB, (ib + 1) * NB)
                jsl = slice(j * P, (j + 1) * P)
                nc.tensor.matmul(
                    SC[:, slot, 0:NB],
                    kpt[0:HD, p, jsl], qpt[0:HD, p, isl],
                    start=True, stop=True,
                )
                nc.tensor.matmul(
                    SC[:, slot, NB:2 * NB],
                    kpt[HD:P, p, jsl], qpt[HD:P, p, isl],
                    start=True, stop=True,
                )

            def exp_j(u, j):
                slot = (u * NJ + j) % 2
                e = epool.tile([P, 1024], f16, name="e_t")
                nc.scalar.activation(e[:], SC[:, slot, :], Exp, scale=0.125)
                e_tiles[(u, j)] = e
                if VARIANT == "debug" and (u, j) == (0, 0):
                    nc.sync.dma_start(dbg["e0"][:], e[:])

            def av_j(u, j):
                p, ib = UNITS[u]
                par = u % 2
                e = e_tiles.pop((u, j))
                for t in range(8):
                    qs, hl = divmod(t, 2)
                    lhsT = e[:, hl * NB + qs * P:hl * NB + (qs + 1) * P]
                    hg = 2 * p + hl
                    # start=True would zero the WHOLE psum bank, wiping the
                    # sibling regions packed in it — accumulate over memset.
                    nc.tensor.matmul(
                        av_reg(par, t),
                        lhsT,
                        vp[:, j, hg * (HD + 1):hg * (HD + 1) + 65],
                        start=False,
                        stop=(j == NJ - 1),
                        skip_group_check=True,
                    )

            def av_zero(u):
                if u % 2 == 0:
                    nc.vector.memset(AVP[:, 0, 0:6 * RS + 65], 0.0)
                    nc.vector.memset(AVP[:, 1, 0:65], 0.0)
                else:
                    nc.vector.memset(AVP[:, 1, RS:6 * RS + 65], 0.0)
                    nc.vector.memset(AVP[:, 2, 0:RS + 65], 0.0)

            def flat(ap):
                return ap.rearrange("p t c -> p (t c)")

            def _recip(out_ap, in_ap):
                # custom-DVE (reciprocal_approx_fast) hits "ISA wrong length"
                # in this container's walrus; plain InstReciprocal on [128,<=8]
                # is cheap enough.
                nc.vector.reciprocal(out_ap, in_ap)

            def _otrans(dst, src):
                if VARIANT == "notrans":
                    nc.sync.dma_start(dst, src.rearrange("a b -> b a"))
                else:
                    nc.sync.dma_start_transpose(dst, src)

            def norm(u):
                p, ib = UNITS[u]
                par = u % 2
                r = rpool.tile([P, 8], f32, name="r_t")
                if par == 0:
                    l0 = AVP[:, 0, 0:7 * RS].rearrange(
                        "p (t c) -> p t c", c=RS)[:, :, 64:65]
                    _recip(r[:, 0:7], flat(l0))
                    _recip(r[:, 7:8], AVP[:, 1, 64:65])
                else:
                    l0 = AVP[:, 1, RS:7 * RS].rearrange(
                        "p (t c) -> p t c", c=RS)[:, :, 64:65]
                    _recip(r[:, 0:6], flat(l0))
                    l1 = AVP[:, 2, 0:2 * RS].rearrange(
                        "p (t c) -> p t c", c=RS)[:, :, 64:65]
                    _recip(r[:, 6:8], flat(l1))
                o_nq = onqpool.tile([P, 4, P], f16, name="onq_t")
                for t in range(8):
                    qs, hl = divmod(t, 2)
                    nc.vector.tensor_scalar_mul(
                        o_nq[:, qs, hl * HD:(hl + 1) * HD],
                        av_reg(par, t)[:, 0:HD],
                        r[:, t:t + 1],
                    )
                if VARIANT == "debug" and u == 0:
                    avdump = onqpool.tile([P, 2, 512], f32, name="avdump_t")
                    nc.vector.tensor_copy(avdump[:], AVP[:, 0:2, :])
                    nc.sync.dma_start(dbg["avp0"][:], avdump[:].rearrange("p a b -> p (a b)"))
                    nc.sync.dma_start(dbg["onq0"][:], o_nq[:].rearrange("p a b -> p (a b)"))
                    nc.sync.dma_start(dbg["r0"][:], r[:])
                for qs in range(4):
                    _otrans(
                        onorm[:, p, ib * NB + qs * P:ib * NB + (qs + 1) * P],
                        o_nq[:, qs, :],
                    )

            # ---- Wo: one mo sub-block of output block n ----
            def wo_mm(n, mo):
                ps = opspool.tile([P, NB], f32, name="ops_t")
                for k2 in range(2):
                    nc.tensor.matmul(
                        ps[:],
                        wo_sb[:, k2, mo * P:(mo + 1) * P],
                        onorm[:, k2, n * NB:(n + 1) * NB],
                        start=(k2 == 0),
                        stop=(k2 == 1),
                    )
                ot = otpool.tile([P, NB], f16, name="ot_t")
                nc.vector.tensor_copy(ot[:], ps[:])
                nc.sync.dma_start(out[mo * P:(mo + 1) * P, n * NB:(n + 1) * NB], ot[:])

            # ---- schedule ----
            def P_(wsb, xd, dst, bcol, m, n):
                return lambda: proj_qk_n(wsb, xd, dst, bcol, m, n)

            def V_(s):
                return lambda: v_block(s)

            def W_(n, mo):
                return lambda: wo_mm(n, mo)

            K = lambda m, n: P_(wk_sb, xk_q, kpt, 2, m, n)
            Q = lambda m, n: P_(wq_sb, xq_q, qpt, 0, m, n)

            # fills keyed by global j-step gj = u*16 + j  (0..127).
            # HARD RULE: the tile framework does NOT reorder; every producer
            # must be EMITTED before its consumer.  v_block(s) before
            # av_j(0, s) at gj=s+1; K(p, n) before scores_j((p,*), 4n);
            # Q(p, ib) before gj=unit*16.
            fills = {
                0: [V_(4)],
                1: [V_(5), K(0, 1)],
                2: [V_(6)],
                3: [V_(7)],
                4: [V_(8), K(0, 2)],
                5: [V_(9)],
                6: [V_(10)],
                7: [V_(11), K(0, 3)],
                8: [V_(12)],
                9: [V_(13)],
                10: [V_(14)],
                11: [V_(15)],
                13: [Q(0, 1)],
                28: [Q(0, 2)],
                34: [K(1, 0)],
                38: [K(1, 1)],
                42: [Q(0, 3)],
                46: [K(1, 2)],
                50: [K(1, 3)],
                54: [Q(1, 0)],
                58: [Q(1, 1)],
                62: [Q(1, 2)],
                66: [Q(1, 3)],
            }
            # Wo blocks n=0..2 spread into units 5..7.  norm(4+n) is emitted
            # inside window gj=(5+n)*16+1 AFTER that window's fills, so wo
            # fills must start at gj >= (5+n)*16+2.
            for n in range(3):
                base = (5 + n) * 16 + 2
                for mo in range(8):
                    fills.setdefault(base + mo * 13 // 8, []).append(W_(n, mo))

            # warmup
            for s in range(4):
                v_block(s)
            K(0, 0)()
            Q(0, 0)()

            # AV runs with lag 2: av(gj-2)'s gate (exp(gj-2)) is already
            # satisfied when it reaches the in-order PE queue, so it never
            # head-of-line-blocks scores(gj+1) whose gate is exp(gj-1).
            for gj in range(128):
                u, j = divmod(gj, NJ)
                if j == 0:
                    av_zero(u)
                scores_j(u, j)
                exp_j(u, j)
                for f in fills.get(gj, []):
                    f()
                if gj >= 2:
                    u2, j2 = divmod(gj - 2, NJ)
                    av_j(u2, j2)
                    if j2 == NJ - 1:
                        norm(u2)

            # tail
            av_j(7, NJ - 2)
            av_j(7, NJ - 1)
            norm(7)
            for mo in range(8):
                wo_mm(3, mo)
            if VARIANT == "debug":
                nc.sync.dma_start(dbg["qpt"][:], qpt[:].rearrange("p a b -> p (a b)"))
                nc.sync.dma_start(dbg["kpt"][:], kpt[:].rearrange("p a b -> p (a b)"))
                nc.sync.dma_start(dbg["vp"][:], vp[:].rearrange("p a b -> p (a b)"))
                nc.sync.dma_start(dbg["onorm"][:], onorm[:].rearrange("p a b -> p (a b)"))

    return nc


def _get_nc():
    if "nc" not in _nc_cache:
        _install_bir_fix()
        _nc_cache["nc"] = _build_nc()
    return _nc_cache["nc"]


# --------------------------------------------------------------------------
# Host wrapper
# --------------------------------------------------------------------------
def run(inputs, trace=False):
    from concourse.bass_utils import run_bass_kernel_spmd

    q = np.asarray(inputs["q"], np.float32)
    k = np.asarray(inputs["k"], np.float32)
    v = np.asarray(inputs["v"], np.float32)
    Wq = np.asarray(inputs["Wq"], np.float32)
    bq = np.asarray(inputs["bq"], np.float32)
    Wk = np.asarray(inputs["Wk"], np.float32)
    bk = np.asarray(inputs["bk"], np.float32)
    Wv = np.asarray(inputs["Wv"], np.float32)
    bv = np.asarray(inputs["bv"], np.float32)
    Wo = np.asarray(inputs["Wo"], np.float32)
    bo = np.asarray(inputs["bo"], np.float32)

    nc = _get_nc()

    xT = {}
    for b in range(2):
        xT[b] = (
            np.ascontiguousarray(q[b].T).astype(np.float16),
            np.ascontiguousarray(k[b].T).astype(np.float16),
            np.ascontiguousarray(v[b].T).astype(np.float16),
        )

    in_maps = []
    for c in range(8):
        b, g = divmod(c, 4)
        sl = slice(g * DL, (g + 1) * DL)
        bias = np.stack(
            [bq[sl][:P], bq[sl][P:], bk[sl][:P], bk[sl][P:]], axis=1
        ).astype(np.float32)
        in_maps.append({
            "xq": xT[b][0],
            "xk": xT[b][1],
            "xv": xT[b][2],
            "wqT": np.ascontiguousarray(Wq[sl, :].T).astype(np.float16),
            "wkT": np.ascontiguousarray(Wk[sl, :].T).astype(np.float16),
            "wvT": np.ascontiguousarray(Wv[sl, :].T).astype(np.float16),
            "woT": np.ascontiguousarray(Wo[:, sl].T).astype(np.float16),
            "bias": bias,
        })

    res = run_bass_kernel_spmd(
        nc, in_maps, core_ids=list(range(8)), trace=trace,
    )
    outs = [r["out"] for r in res.results]

    const = (Wo @ bv + bo).astype(np.float32)  # [1024]
    full = np.empty((2, S, H), np.float32)
    for b in range(2):
        acc = outs[4 * b].astype(np.float32).copy()
        for g in range(1, 4):
            acc += outs[4 * b + g]
        full[b] = acc.T + const
    return full, res


def kernel(**inputs):
    full, _ = run(inputs, trace=False)
    return full


# revision 26
# speedup vs baseline: 1.2752x; 1.2698x over previous
"""Multi-head attention (B=2, S=2048, H=1024, 16 heads) on 8 TRN2 NeuronCores.

Sharding: data-parallel over batch (2) x tensor-parallel over heads (16 -> 4
groups of 4 heads).  Core c = b*4 + g handles batch b, heads [4g, 4g+4).

v2 structure (fp16 storage / fp32 accumulate), x = q|k|v of its batch:
  QP_T[d, s] = (Wq_g x^T + bq_g)   d on partitions
  KP_T[d, s] = (Wk_g x^T + bk_g)
  VP[s, d]   = x Wv_g^T            natural layout + ones column per head
  per (pair p, i-block, j-tile):
     S_T[j, i] = K^T-contracted scores  (two heads share the PE via row tiles)
     E = exp(S_T/8)  on ACT, fused over two j-tiles  ([128, 2048] per instr)
     O[q, d+1] += E_j^T @ [V_j | 1]    transposed AV: q on psum partitions,
                                       ones column accumulates the denom L
  norm: r = 1/L (custom-DVE fast reciprocal), O_n = O * r (tensor_scalar),
        O_n^T via DMA-transpose (xbar) -> onorm[d, q]
  out_T[o, i] = Wo_g^T-contracted projection of onorm -> [1024, 2048] f32->f16
Host: out[b] = sum_g out_T(b,g)^T + (Wo @ bv + bo).
"""

import json

import numpy as np

S = 2048
H = 1024
DL = 256          # local projection dim = 4 heads * 64
P = 128
HD = 64
NK = H // P       # 8 k-tiles over hidden dim
NB = 512          # i-block (queries per attention unit column block)
NJ = S // P       # 16 j tiles of 128 keys
VW = 4 * (HD + 1)  # vp row width: 4 heads x (64 + ones col)
RS = 72           # AV psum region stride (32B aligned); region width 65

_nc_cache = {}
VARIANT = "full"  # debug bisect hook


# --------------------------------------------------------------------------
# BIR fix: this container's walrus supports only ONE sync wait (and update)
# per TPB instruction; Tile attaches several.  Split extras onto single-wait
# EventSemaphore instructions at the serialization boundary.
# --------------------------------------------------------------------------
_wsplit_counter = [0]


def _mk_evsem(engine, debug, wait=None, update=None):
    _wsplit_counter[0] += 1
    return {
        "debug": debug,
        "engine": engine,
        "ins": [],
        "outs": [],
        "name": f"wsplit-{_wsplit_counter[0]}",
        "opcode": "EventSemaphore",
        "sync_info": {
            "on_wait": [wait] if wait else [],
            "on_update": [update] if update else [],
        },
    }


def _split_bir_waits(bir):
    for f in bir.get("functions", []):
        for blk in f.get("blocks", []):
            out = []
            for inst in blk.get("instructions", []):
                si = inst.get("sync_info")
                waits = list(si.get("on_wait") or []) if si else []
                updates = list(si.get("on_update") or []) if si else []
                eng = inst.get("engine")
                dbg = inst.get("debug", 0)
                if len(waits) > 1:
                    for w in waits[:-1]:
                        out.append(_mk_evsem(eng, dbg, wait=w))
                    si["on_wait"] = [waits[-1]]
                out.append(inst)
                if len(updates) > 1:
                    si["on_update"] = [updates[0]]
                    for u in updates[1:]:
                        out.append(_mk_evsem(eng, dbg, update=u))
            blk["instructions"] = out
    return bir


def _install_bir_fix():
    import concourse.bass as bass

    if getattr(bass.Bass, "_wsplit_installed", False):
        return
    orig = bass.Bass.to_json_bytes

    def to_json_bytes(self, *a, **k):
        bir = json.loads(orig(self, *a, **k))
        return json.dumps(_split_bir_waits(bir)).encode()

    bass.Bass.to_json_bytes = to_json_bytes
    bass.Bass._wsplit_installed = True


# --------------------------------------------------------------------------
# Kernel builder
# --------------------------------------------------------------------------

def _build_nc():
    import concourse.bass as bass
    import concourse.mybir as mybir
    import concourse.tile as tile

    f16 = mybir.dt.float16
    f32 = mybir.dt.float32
    Exp = mybir.ActivationFunctionType.Exp

    nc = bass.Bass("TRN2")

    xq = nc.dram_tensor("xq", [H, S], f16, kind="ExternalInput")
    xk = nc.dram_tensor("xk", [H, S], f16, kind="ExternalInput")
    xv = nc.dram_tensor("xv", [H, S], f16, kind="ExternalInput")
    wqT = nc.dram_tensor("wqT", [H, DL], f16, kind="ExternalInput")
    wkT = nc.dram_tensor("wkT", [H, DL], f16, kind="ExternalInput")
    wvT = nc.dram_tensor("wvT", [H, DL], f16, kind="ExternalInput")
    woT = nc.dram_tensor("woT", [DL, H], f16, kind="ExternalInput")
    bias = nc.dram_tensor("bias", [P, 4], f32, kind="ExternalInput")  # bq0 bq1 bk0 bk1
    out = nc.dram_tensor("out", [H, S], f16, kind="ExternalOutput")
    dbg = {}
    if VARIANT == "debug":
        dbg["qpt"] = nc.dram_tensor("dbg_qpt", [P, 2 * S], f16, kind="ExternalOutput")
        dbg["kpt"] = nc.dram_tensor("dbg_kpt", [P, 2 * S], f16, kind="ExternalOutput")
        dbg["vp"] = nc.dram_tensor("dbg_vp", [P, NJ * VW], f16, kind="ExternalOutput")
        dbg["e0"] = nc.dram_tensor("dbg_e0", [P, 1024], f16, kind="ExternalOutput")
        dbg["onq0"] = nc.dram_tensor("dbg_onq0", [P, 4 * P], f16, kind="ExternalOutput")
        dbg["onorm"] = nc.dram_tensor("dbg_onorm", [P, 2 * S], f16, kind="ExternalOutput")
        dbg["r0"] = nc.dram_tensor("dbg_r0", [P, 8], f32, kind="ExternalOutput")
        dbg["avp0"] = nc.dram_tensor("dbg_avp0", [P, 2 * 512], f32, kind="ExternalOutput")

    UNITS = [(p, ib) for p in range(2) for ib in range(4)]  # p-major

    with tile.TileContext(nc) as tc:
        with (
            tc.tile_pool(name="persist", bufs=1) as persist,
            tc.tile_pool(name="xpool", bufs=96) as xpool,
            tc.tile_pool(name="epool", bufs=6) as epool,
            tc.tile_pool(name="onq", bufs=2) as onqpool,
            tc.tile_pool(name="rp", bufs=2) as rpool,
            tc.tile_pool(name="otp", bufs=3) as otpool,
            tc.tile_pool(name="scp", bufs=2, space="PSUM") as scpool,
            tc.tile_pool(name="avp", bufs=1, space="PSUM") as avpool,
            tc.tile_pool(name="opsp", bufs=1, space="PSUM") as opspool,
        ):
            # ---- persistent tiles ----
            wq_sb = persist.tile([P, NK, DL], f16, name="wq_sb")
            wk_sb = persist.tile([P, NK, DL], f16, name="wk_sb")
            wv_sb = persist.tile([P, NK, DL], f16, name="wv_sb")
            wo_sb = persist.tile([P, 2, H], f16, name="wo_sb")
            bias_sb = persist.tile([P, 4], f32, name="bias_sb")
            qpt = persist.tile([P, 2, S], f16, name="qpt")
            kpt = persist.tile([P, 2, S], f16, name="kpt")
            vp = persist.tile([P, NJ, VW], f16, name="vp")
            onorm = persist.tile([P, 2, S], f16, name="onorm")

            # psum: SC slots as pool-rotated tiles (2x2 banks; psum deps are
            # whole-tile, so slots must be separate tiles) + AV ping-pong
            # packed regions (3 banks)
            AVP = avpool.tile([P, 3, 512], f32, name="AVP")

            # ---- DMA loads, first-needed first ----
            nc.sync.dma_start(wv_sb[:], wvT.rearrange("(k p) d -> p k d", p=P))
            xv_q = {}
            xk_q = {}
            xq_q = {}

            def load_q(dst, src, k, n):
                t = xpool.tile([P, NB], f16, name="x_sb")
                nc.sync.dma_start(t[:], src[k * P:(k + 1) * P, n * NB:(n + 1) * NB])
                dst[(k, n)] = t

            for k in range(NK):
                load_q(xv_q, xv, k, 0)
            nc.sync.dma_start(wk_sb[:], wkT.rearrange("(k p) d -> p k d", p=P))
            for k in range(NK):
                load_q(xk_q, xk, k, 0)
            nc.sync.dma_start(wq_sb[:], wqT.rearrange("(k p) d -> p k d", p=P))
            for k in range(NK):
                load_q(xq_q, xq, k, 0)
            nc.sync.dma_start(bias_sb[:], bias[:])
            for n in range(1, 4):
                for k in range(NK):
                    load_q(xk_q, xk, k, n)
                for k in range(NK):
                    load_q(xv_q, xv, k, n)
                for k in range(NK):
                    load_q(xq_q, xq, k, n)
            nc.sync.dma_start(wo_sb[:], woT.rearrange("(k p) d -> p k d", p=P))

            # ones columns in vp (col 64 of each head block)
            for h in range(4):
                nc.gpsimd.memset(vp[:, :, h * (HD + 1) + HD:h * (HD + 1) + HD + 1], 1.0)

            # ---- AV psum region map: parity ping-pong over 3 banks ----
            def av_reg(par, t):
                if par == 0:
                    if t < 7:
                        return AVP[:, 0, RS * t:RS * t + 65]
                    return AVP[:, 1, 0:65]
                else:
                    if t < 6:
                        return AVP[:, 1, RS + RS * t:RS + RS * t + 65]
                    return AVP[:, 2, RS * (t - 6):RS * (t - 6) + 65]

            # ---- V projection for one s-tile ----
            def v_block(s):
                n, c = divmod(s, 4)
                ps = opspool.tile([P, NB], f32, name="ops_t")
                for k in range(NK):
                    nc.tensor.matmul(
                        ps[:, :DL],
                        xv_q[(k, n)][:, c * P:(c + 1) * P],
                        wv_sb[:, k, :],
                        start=(k == 0),
                        stop=(k == NK - 1),
                    )
                for hg in range(4):
                    nc.vector.tensor_copy(
                        vp[:, s, hg * (HD + 1):hg * (HD + 1) + HD],
                        ps[:, hg * HD:(hg + 1) * HD],
                    )

            # ---- K/Q projection, one (m, n) block ----
            def proj_qk_n(wsb, xq_, dst, bcol, m, n):
                ps = opspool.tile([P, NB], f32, name="ops_t")
                for k in range(NK):
                    nc.tensor.matmul(
                        ps[:],
                        wsb[:, k, m * P:(m + 1) * P],
                        xq_[(k, n)][:],
                        start=(k == 0),
                        stop=(k == NK - 1),
                    )
                nc.vector.tensor_scalar_add(
                    dst[:, m, n * NB:(n + 1) * NB], ps[:],
                    bias_sb[:, bcol + m:bcol + m + 1],
                )

            # ---- attention pieces ----
            e_tiles = {}
            sc_tiles = {}

            def scores_j(u, j):
                p, ib = UNITS[u]
                isl = slice(ib * NB, (ib + 1) * NB)
                jsl = slice(j * P, (j + 1) * P)
                sc = scpool.tile([P, 1024], f32, name="sc_t")
                nc.tensor.matmul(
                    sc[:, 0:NB],
                    kpt[0:HD, p, jsl], qpt[0:HD, p, isl],
                    start=True, stop=True,
                )
                nc.tensor.matmul(
                    sc[:, NB:2 * NB],
                    kpt[HD:P, p, jsl], qpt[HD:P, p, isl],
                    start=True, stop=True,
                )
                sc_tiles[(u, j)] = sc

            def exp_j(u, j):
                sc = sc_tiles.pop((u, j))
                e = epool.tile([P, 1024], f16, name="e_t")
                nc.scalar.activation(e[:], sc[:], Exp, scale=0.125)
                e_tiles[(u, j)] = e
                if VARIANT == "debug" and (u, j) == (0, 0):
                    nc.sync.dma_start(dbg["e0"][:], e[:])

            def av_j(u, j):
                p, ib = UNITS[u]
                par = u % 2
                e = e_tiles.pop((u, j))
                for t in range(8):
                    qs, hl = divmod(t, 2)
                    lhsT = e[:, hl * NB + qs * P:hl * NB + (qs + 1) * P]
                    hg = 2 * p + hl
                    # start=True would zero the WHOLE psum bank, wiping the
                    # sibling regions packed in it — accumulate over memset.
                    nc.tensor.matmul(
                        av_reg(par, t),
                        lhsT,
                        vp[:, j, hg * (HD + 1):hg * (HD + 1) + 65],
                        start=False,
                        stop=(j == NJ - 1),
                        skip_group_check=True,
                    )

            def av_zero(u):
                if u % 2 == 0:
                    nc.vector.memset(AVP[:, 0, 0:6 * RS + 65], 0.0)
                    nc.vector.memset(AVP[:, 1, 0:65], 0.0)
                else:
                    nc.vector.memset(AVP[:, 1, RS:6 * RS + 65], 0.0)
                    nc.vector.memset(AVP[:, 2, 0:RS + 65], 0.0)

            def flat(ap):
                return ap.rearrange("p t c -> p (t c)")

            def _recip(out_ap, in_ap):
                # custom-DVE (reciprocal_approx_fast) hits "ISA wrong length"
                # in this container's walrus; plain InstReciprocal on [128,<=8]
                # is cheap enough.
                nc.vector.reciprocal(out_ap, in_ap)

            def _otrans(dst, src):
                if VARIANT == "notrans":
                    nc.sync.dma_start(dst, src.rearrange("a b -> b a"))
                else:
                    nc.sync.dma_start_transpose(dst, src)

            def norm(u):
                p, ib = UNITS[u]
                par = u % 2
                r = rpool.tile([P, 8], f32, name="r_t")
                if par == 0:
                    l0 = AVP[:, 0, 0:7 * RS].rearrange(
                        "p (t c) -> p t c", c=RS)[:, :, 64:65]
                    _recip(r[:, 0:7], flat(l0))
                    _recip(r[:, 7:8], AVP[:, 1, 64:65])
                else:
                    l0 = AVP[:, 1, RS:7 * RS].rearrange(
                        "p (t c) -> p t c", c=RS)[:, :, 64:65]
                    _recip(r[:, 0:6], flat(l0))
                    l1 = AVP[:, 2, 0:2 * RS].rearrange(
                        "p (t c) -> p t c", c=RS)[:, :, 64:65]
                    _recip(r[:, 6:8], flat(l1))
                o_nq = onqpool.tile([P, 4, P], f16, name="onq_t")
                for t in range(8):
                    qs, hl = divmod(t, 2)
                    nc.vector.tensor_scalar_mul(
                        o_nq[:, qs, hl * HD:(hl + 1) * HD],
                        av_reg(par, t)[:, 0:HD],
                        r[:, t:t + 1],
                    )
                if VARIANT == "debug" and u == 0:
                    avdump = onqpool.tile([P, 2, 512], f32, name="avdump_t")
                    nc.vector.tensor_copy(avdump[:], AVP[:, 0:2, :])
                    nc.sync.dma_start(dbg["avp0"][:], avdump[:].rearrange("p a b -> p (a b)"))
                    nc.sync.dma_start(dbg["onq0"][:], o_nq[:].rearrange("p a b -> p (a b)"))
                    nc.sync.dma_start(dbg["r0"][:], r[:])
                for qs in range(4):
                    _otrans(
                        onorm[:, p, ib * NB + qs * P:ib * NB + (qs + 1) * P],
                        o_nq[:, qs, :],
                    )

            # ---- Wo: one mo sub-block of output block n ----
            def wo_mm(n, mo):
                ps = opspool.tile([P, NB], f32, name="ops_t")
                for k2 in range(2):
                    nc.tensor.matmul(
                        ps[:],
                        wo_sb[:, k2, mo * P:(mo + 1) * P],
                        onorm[:, k2, n * NB:(n + 1) * NB],
                        start=(k2 == 0),
                        stop=(k2 == 1),
                    )
                ot = otpool.tile([P, NB], f16, name="ot_t")
                nc.vector.tensor_copy(ot[:], ps[:])
                nc.sync.dma_start(out[mo * P:(mo + 1) * P, n * NB:(n + 1) * NB], ot[:])

            # ---- schedule ----
            def P_(wsb, xd, dst, bcol, m, n):
                return lambda: proj_qk_n(wsb, xd, dst, bcol, m, n)

            def V_(s):
                return lambda: v_block(s)

            def W_(n, mo):
                return lambda: wo_mm(n, mo)

            K = lambda m, n: P_(wk_sb, xk_q, kpt, 2, m, n)
            Q = lambda m, n: P_(wq_sb, xq_q, qpt, 0, m, n)

            # fills keyed by global j-step gj = u*16 + j  (0..127).
            # HARD RULE: the tile framework does NOT reorder; every producer
            # must be EMITTED before its consumer.  v_block(s) before
            # av_j(0, s) at gj=s+1; K(p, n) before scores_j((p,*), 4n);
            # Q(p, ib) before gj=unit*16.
            fills = {
                0: [V_(4)],
                1: [V_(5), K(0, 1)],
                2: [V_(6)],
                3: [V_(7)],
                4: [V_(8), K(0, 2)],
                5: [V_(9)],
                6: [V_(10)],
                7: [V_(11), K(0, 3)],
                8: [V_(12)],
                9: [V_(13)],
                10: [V_(14)],
                11: [V_(15)],
                13: [Q(0, 1)],
                28: [Q(0, 2)],
                34: [K(1, 0)],
                38: [K(1, 1)],
                42: [Q(0, 3)],
                46: [K(1, 2)],
                50: [K(1, 3)],
                54: [Q(1, 0)],
                58: [Q(1, 1)],
                62: [Q(1, 2)],
                66: [Q(1, 3)],
            }
            # Wo blocks n=0..2 spread into units 5..7.  norm(4+n) is emitted
            # inside window gj=(5+n)*16+1 AFTER that window's fills, so wo
            # fills must start at gj >= (5+n)*16+2.
            for n in range(3):
                base = (5 + n) * 16 + 2
                for mo in range(8):
                    fills.setdefault(base + mo * 13 // 8, []).append(W_(n, mo))

            # warmup
            for s in range(4):
                v_block(s)
            K(0, 0)()
            Q(0, 0)()

            # AV runs with lag 2: av(gj-2)'s gate (exp(gj-2)) is already
            # satisfied when it reaches the in-order PE queue, so it never
            # head-of-line-blocks scores(gj+1) whose gate is exp(gj-1).
            for gj in range(128):
                u, j = divmod(gj, NJ)
                if j == 0:
                    av_zero(u)
                scores_j(u, j)
                exp_j(u, j)
                if gj >= 2:
                    u2, j2 = divmod(gj - 2, NJ)
                    av_j(u2, j2)
                    if j2 == NJ - 1:
                        norm(u2)
                for f in fills.get(gj, []):
                    f()

            # tail
            av_j(7, NJ - 2)
            av_j(7, NJ - 1)
            norm(7)
            for mo in range(8):
                wo_mm(3, mo)
            if VARIANT == "debug":
                nc.sync.dma_start(dbg["qpt"][:], qpt[:].rearrange("p a b -> p (a b)"))
                nc.sync.dma_start(dbg["kpt"][:], kpt[:].rearrange("p a b -> p (a b)"))
                nc.sync.dma_start(dbg["vp"][:], vp[:].rearrange("p a b -> p (a b)"))
                nc.sync.dma_start(dbg["onorm"][:], onorm[:].rearrange("p a b -> p (a b)"))

    return nc


def _get_nc():
    if "nc" not in _nc_cache:
        _install_bir_fix()
        _nc_cache["nc"] = _build_nc()
    return _nc_cache["nc"]


# --------------------------------------------------------------------------
# Host wrapper
# --------------------------------------------------------------------------
def run(inputs, trace=False):
    from concourse.bass_utils import run_bass_kernel_spmd

    q = np.asarray(inputs["q"], np.float32)
    k = np.asarray(inputs["k"], np.float32)
    v = np.asarray(inputs["v"], np.float32)
    Wq = np.asarray(inputs["Wq"], np.float32)
    bq = np.asarray(inputs["bq"], np.float32)
    Wk = np.asarray(inputs["Wk"], np.float32)
    bk = np.asarray(inputs["bk"], np.float32)
    Wv = np.asarray(inputs["Wv"], np.float32)
    bv = np.asarray(inputs["bv"], np.float32)
    Wo = np.asarray(inputs["Wo"], np.float32)
    bo = np.asarray(inputs["bo"], np.float32)

    nc = _get_nc()

    xT = {}
    for b in range(2):
        xT[b] = (
            np.ascontiguousarray(q[b].T).astype(np.float16),
            np.ascontiguousarray(k[b].T).astype(np.float16),
            np.ascontiguousarray(v[b].T).astype(np.float16),
        )

    in_maps = []
    for c in range(8):
        b, g = divmod(c, 4)
        sl = slice(g * DL, (g + 1) * DL)
        bias = np.stack(
            [bq[sl][:P], bq[sl][P:], bk[sl][:P], bk[sl][P:]], axis=1
        ).astype(np.float32)
        in_maps.append({
            "xq": xT[b][0],
            "xk": xT[b][1],
            "xv": xT[b][2],
            "wqT": np.ascontiguousarray(Wq[sl, :].T).astype(np.float16),
            "wkT": np.ascontiguousarray(Wk[sl, :].T).astype(np.float16),
            "wvT": np.ascontiguousarray(Wv[sl, :].T).astype(np.float16),
            "woT": np.ascontiguousarray(Wo[:, sl].T).astype(np.float16),
            "bias": bias,
        })

    res = run_bass_kernel_spmd(
        nc, in_maps, core_ids=list(range(8)), trace=trace,
    )
    outs = [r["out"] for r in res.results]

    const = (Wo @ bv + bo).astype(np.float32)  # [1024]
    full = np.empty((2, S, H), np.float32)
    for b in range(2):
        acc = outs[4 * b].astype(np.float32).copy()
        for g in range(1, 4):
            acc += outs[4 * b + g]
        full[b] = acc.T + const
    return full, res


def kernel(**inputs):
    full, _ = run(inputs, trace=False)
    return full


# revision 27
# speedup vs baseline: 1.2932x; 1.0141x over previous
"""Multi-head attention (B=2, S=2048, H=1024, 16 heads) on 8 TRN2 NeuronCores.

Sharding: data-parallel over batch (2) x tensor-parallel over heads (16 -> 4
groups of 4 heads).  Core c = b*4 + g handles batch b, heads [4g, 4g+4).

v2 structure (fp16 storage / fp32 accumulate), x = q|k|v of its batch:
  QP_T[d, s] = (Wq_g x^T + bq_g)   d on partitions
  KP_T[d, s] = (Wk_g x^T + bk_g)
  VP[s, d]   = x Wv_g^T            natural layout + ones column per head
  per (pair p, i-block, j-tile):
     S_T[j, i] = K^T-contracted scores  (two heads share the PE via row tiles)
     E = exp(S_T/8)  on ACT, fused over two j-tiles  ([128, 2048] per instr)
     O[q, d+1] += E_j^T @ [V_j | 1]    transposed AV: q on psum partitions,
                                       ones column accumulates the denom L
  norm: r = 1/L (custom-DVE fast reciprocal), O_n = O * r (tensor_scalar),
        O_n^T via DMA-transpose (xbar) -> onorm[d, q]
  out_T[o, i] = Wo_g^T-contracted projection of onorm -> [1024, 2048] f32->f16
Host: out[b] = sum_g out_T(b,g)^T + (Wo @ bv + bo).
"""

import json

import numpy as np

S = 2048
H = 1024
DL = 256          # local projection dim = 4 heads * 64
P = 128
HD = 64
NK = H // P       # 8 k-tiles over hidden dim
NB = 512          # i-block (queries per attention unit column block)
NJ = S // P       # 16 j tiles of 128 keys
VW = 4 * (HD + 1)  # vp row width: 4 heads x (64 + ones col)
RS = 72           # AV psum region stride (32B aligned); region width 65

_nc_cache = {}
VARIANT = "full"  # debug bisect hook


# --------------------------------------------------------------------------
# BIR fix: this container's walrus supports only ONE sync wait (and update)
# per TPB instruction; Tile attaches several.  Split extras onto single-wait
# EventSemaphore instructions at the serialization boundary.
# --------------------------------------------------------------------------
_wsplit_counter = [0]


def _mk_evsem(engine, debug, wait=None, update=None):
    _wsplit_counter[0] += 1
    return {
        "debug": debug,
        "engine": engine,
        "ins": [],
        "outs": [],
        "name": f"wsplit-{_wsplit_counter[0]}",
        "opcode": "EventSemaphore",
        "sync_info": {
            "on_wait": [wait] if wait else [],
            "on_update": [update] if update else [],
        },
    }


def _split_bir_waits(bir):
    for f in bir.get("functions", []):
        for blk in f.get("blocks", []):
            out = []
            for inst in blk.get("instructions", []):
                si = inst.get("sync_info")
                waits = list(si.get("on_wait") or []) if si else []
                updates = list(si.get("on_update") or []) if si else []
                eng = inst.get("engine")
                dbg = inst.get("debug", 0)
                if len(waits) > 1:
                    for w in waits[:-1]:
                        out.append(_mk_evsem(eng, dbg, wait=w))
                    si["on_wait"] = [waits[-1]]
                out.append(inst)
                if len(updates) > 1:
                    si["on_update"] = [updates[0]]
                    for u in updates[1:]:
                        out.append(_mk_evsem(eng, dbg, update=u))
            blk["instructions"] = out
    return bir


def _install_bir_fix():
    import concourse.bass as bass

    if getattr(bass.Bass, "_wsplit_installed", False):
        return
    orig = bass.Bass.to_json_bytes

    def to_json_bytes(self, *a, **k):
        bir = json.loads(orig(self, *a, **k))
        return json.dumps(_split_bir_waits(bir)).encode()

    bass.Bass.to_json_bytes = to_json_bytes
    bass.Bass._wsplit_installed = True


# --------------------------------------------------------------------------
# Kernel builder
# --------------------------------------------------------------------------

def _build_nc():
    import concourse.bass as bass
    import concourse.mybir as mybir
    import concourse.tile as tile

    f16 = mybir.dt.float16
    f32 = mybir.dt.float32
    Exp = mybir.ActivationFunctionType.Exp

    nc = bass.Bass("TRN2")

    xq = nc.dram_tensor("xq", [H, S], f16, kind="ExternalInput")
    xk = nc.dram_tensor("xk", [H, S], f16, kind="ExternalInput")
    xv = nc.dram_tensor("xv", [H, S], f16, kind="ExternalInput")
    wqT = nc.dram_tensor("wqT", [H, DL], f16, kind="ExternalInput")
    wkT = nc.dram_tensor("wkT", [H, DL], f16, kind="ExternalInput")
    wvT = nc.dram_tensor("wvT", [H, DL], f16, kind="ExternalInput")
    woT = nc.dram_tensor("woT", [DL, H], f16, kind="ExternalInput")
    bias = nc.dram_tensor("bias", [P, 4], f32, kind="ExternalInput")  # bq0 bq1 bk0 bk1
    out = nc.dram_tensor("out", [H, S], f16, kind="ExternalOutput")
    dbg = {}
    if VARIANT == "debug":
        dbg["qpt"] = nc.dram_tensor("dbg_qpt", [P, 2 * S], f16, kind="ExternalOutput")
        dbg["kpt"] = nc.dram_tensor("dbg_kpt", [P, 2 * S], f16, kind="ExternalOutput")
        dbg["vp"] = nc.dram_tensor("dbg_vp", [P, NJ * VW], f16, kind="ExternalOutput")
        dbg["e0"] = nc.dram_tensor("dbg_e0", [P, 1024], f16, kind="ExternalOutput")
        dbg["onq0"] = nc.dram_tensor("dbg_onq0", [P, 4 * P], f16, kind="ExternalOutput")
        dbg["onorm"] = nc.dram_tensor("dbg_onorm", [P, 2 * S], f16, kind="ExternalOutput")
        dbg["r0"] = nc.dram_tensor("dbg_r0", [P, 8], f32, kind="ExternalOutput")
        dbg["avp0"] = nc.dram_tensor("dbg_avp0", [P, 2 * 512], f32, kind="ExternalOutput")

    UNITS = [(p, ib) for p in range(2) for ib in range(4)]  # p-major

    with tile.TileContext(nc) as tc:
        with (
            tc.tile_pool(name="persist", bufs=1) as persist,
            tc.tile_pool(name="xpool", bufs=96) as xpool,
            tc.tile_pool(name="epool", bufs=10) as epool,
            tc.tile_pool(name="onq", bufs=2) as onqpool,
            tc.tile_pool(name="rp", bufs=2) as rpool,
            tc.tile_pool(name="otp", bufs=3) as otpool,
            tc.tile_pool(name="scp", bufs=2, space="PSUM") as scpool,
            tc.tile_pool(name="avp", bufs=1, space="PSUM") as avpool,
            tc.tile_pool(name="opsp", bufs=1, space="PSUM") as opspool,
        ):
            # ---- persistent tiles ----
            wq_sb = persist.tile([P, NK, DL], f16, name="wq_sb")
            wk_sb = persist.tile([P, NK, DL], f16, name="wk_sb")
            wv_sb = persist.tile([P, NK, DL], f16, name="wv_sb")
            wo_sb = persist.tile([P, 2, H], f16, name="wo_sb")
            bias_sb = persist.tile([P, 4], f32, name="bias_sb")
            qpt = persist.tile([P, 2, S], f16, name="qpt")
            kpt = persist.tile([P, 2, S], f16, name="kpt")
            vp = persist.tile([P, NJ, VW], f16, name="vp")
            onorm = persist.tile([P, 2, S], f16, name="onorm")

            # psum: SC slots as pool-rotated tiles (2x2 banks; psum deps are
            # whole-tile, so slots must be separate tiles) + AV ping-pong
            # packed regions (3 banks)
            AVP = avpool.tile([P, 3, 512], f32, name="AVP")

            # ---- DMA loads, first-needed first ----
            nc.sync.dma_start(wv_sb[:], wvT.rearrange("(k p) d -> p k d", p=P))
            xv_q = {}
            xk_q = {}
            xq_q = {}

            def load_q(dst, src, k, n):
                t = xpool.tile([P, NB], f16, name="x_sb")
                nc.sync.dma_start(t[:], src[k * P:(k + 1) * P, n * NB:(n + 1) * NB])
                dst[(k, n)] = t

            for k in range(NK):
                load_q(xv_q, xv, k, 0)
            nc.sync.dma_start(wk_sb[:], wkT.rearrange("(k p) d -> p k d", p=P))
            for k in range(NK):
                load_q(xk_q, xk, k, 0)
            nc.sync.dma_start(wq_sb[:], wqT.rearrange("(k p) d -> p k d", p=P))
            for k in range(NK):
                load_q(xq_q, xq, k, 0)
            nc.sync.dma_start(bias_sb[:], bias[:])
            for n in range(1, 4):
                for k in range(NK):
                    load_q(xk_q, xk, k, n)
                for k in range(NK):
                    load_q(xv_q, xv, k, n)
                for k in range(NK):
                    load_q(xq_q, xq, k, n)
            nc.sync.dma_start(wo_sb[:], woT.rearrange("(k p) d -> p k d", p=P))

            # ones columns in vp (col 64 of each head block)
            for h in range(4):
                nc.gpsimd.memset(vp[:, :, h * (HD + 1) + HD:h * (HD + 1) + HD + 1], 1.0)

            # ---- AV psum region map: parity ping-pong over 3 banks ----
            def av_reg(par, t):
                if par == 0:
                    if t < 7:
                        return AVP[:, 0, RS * t:RS * t + 65]
                    return AVP[:, 1, 0:65]
                else:
                    if t < 6:
                        return AVP[:, 1, RS + RS * t:RS + RS * t + 65]
                    return AVP[:, 2, RS * (t - 6):RS * (t - 6) + 65]

            # ---- V projection for one s-tile ----
            def v_block(s, pool=None):
                n, c = divmod(s, 4)
                if pool is None:
                    ps = opspool.tile([P, NB], f32, name="ops_t")
                else:
                    ps = pool.tile([P, 1024], f32, name="sc_t")[:, 0:NB]
                for k in range(NK):
                    nc.tensor.matmul(
                        ps[:, :DL],
                        xv_q[(k, n)][:, c * P:(c + 1) * P],
                        wv_sb[:, k, :],
                        start=(k == 0),
                        stop=(k == NK - 1),
                    )
                for hg in range(4):
                    nc.vector.tensor_copy(
                        vp[:, s, hg * (HD + 1):hg * (HD + 1) + HD],
                        ps[:, hg * HD:(hg + 1) * HD],
                    )

            # ---- K/Q projection, one (m, n) block ----
            def proj_qk_n(wsb, xq_, dst, bcol, m, n, pool=None):
                if pool is None:
                    ps = opspool.tile([P, NB], f32, name="ops_t")
                else:
                    ps = pool.tile([P, 1024], f32, name="sc_t")[:, 0:NB]
                for k in range(NK):
                    nc.tensor.matmul(
                        ps[:],
                        wsb[:, k, m * P:(m + 1) * P],
                        xq_[(k, n)][:],
                        start=(k == 0),
                        stop=(k == NK - 1),
                    )
                nc.vector.tensor_scalar_add(
                    dst[:, m, n * NB:(n + 1) * NB], ps[:],
                    bias_sb[:, bcol + m:bcol + m + 1],
                )

            # ---- attention pieces ----
            e_tiles = {}
            sc_tiles = {}

            def scores_j(u, j):
                p, ib = UNITS[u]
                isl = slice(ib * NB, (ib + 1) * NB)
                jsl = slice(j * P, (j + 1) * P)
                sc = scpool.tile([P, 1024], f32, name="sc_t")
                nc.tensor.matmul(
                    sc[:, 0:NB],
                    kpt[0:HD, p, jsl], qpt[0:HD, p, isl],
                    start=True, stop=True,
                )
                nc.tensor.matmul(
                    sc[:, NB:2 * NB],
                    kpt[HD:P, p, jsl], qpt[HD:P, p, isl],
                    start=True, stop=True,
                )
                sc_tiles[(u, j)] = sc

            def exp_j(u, j):
                sc = sc_tiles.pop((u, j))
                e = epool.tile([P, 1024], f16, name="e_t")
                nc.scalar.activation(e[:], sc[:], Exp, scale=0.125)
                e_tiles[(u, j)] = e
                if VARIANT == "debug" and (u, j) == (0, 0):
                    nc.sync.dma_start(dbg["e0"][:], e[:])

            def av_j(u, j):
                p, ib = UNITS[u]
                par = u % 2
                e = e_tiles.pop((u, j))
                for t in range(8):
                    qs, hl = divmod(t, 2)
                    lhsT = e[:, hl * NB + qs * P:hl * NB + (qs + 1) * P]
                    hg = 2 * p + hl
                    # start=True would zero the WHOLE psum bank, wiping the
                    # sibling regions packed in it — accumulate over memset.
                    nc.tensor.matmul(
                        av_reg(par, t),
                        lhsT,
                        vp[:, j, hg * (HD + 1):hg * (HD + 1) + 65],
                        start=False,
                        stop=(j == NJ - 1),
                        skip_group_check=True,
                    )

            def av_zero(u):
                if u % 2 == 0:
                    nc.vector.memset(AVP[:, 0, 0:6 * RS + 65], 0.0)
                    nc.vector.memset(AVP[:, 1, 0:65], 0.0)
                else:
                    nc.vector.memset(AVP[:, 1, RS:6 * RS + 65], 0.0)
                    nc.vector.memset(AVP[:, 2, 0:RS + 65], 0.0)

            def flat(ap):
                return ap.rearrange("p t c -> p (t c)")

            def _recip(out_ap, in_ap):
                # custom-DVE (reciprocal_approx_fast) hits "ISA wrong length"
                # in this container's walrus; plain InstReciprocal on [128,<=8]
                # is cheap enough.
                nc.vector.reciprocal(out_ap, in_ap)

            def _otrans(dst, src):
                if VARIANT == "notrans":
                    nc.sync.dma_start(dst, src.rearrange("a b -> b a"))
                else:
                    nc.sync.dma_start_transpose(dst, src)

            def norm(u):
                p, ib = UNITS[u]
                par = u % 2
                r = rpool.tile([P, 8], f32, name="r_t")
                if par == 0:
                    l0 = AVP[:, 0, 0:7 * RS].rearrange(
                        "p (t c) -> p t c", c=RS)[:, :, 64:65]
                    _recip(r[:, 0:7], flat(l0))
                    _recip(r[:, 7:8], AVP[:, 1, 64:65])
                else:
                    l0 = AVP[:, 1, RS:7 * RS].rearrange(
                        "p (t c) -> p t c", c=RS)[:, :, 64:65]
                    _recip(r[:, 0:6], flat(l0))
                    l1 = AVP[:, 2, 0:2 * RS].rearrange(
                        "p (t c) -> p t c", c=RS)[:, :, 64:65]
                    _recip(r[:, 6:8], flat(l1))
                o_nq = onqpool.tile([P, 4, P], f16, name="onq_t")
                for t in range(8):
                    qs, hl = divmod(t, 2)
                    nc.vector.tensor_scalar_mul(
                        o_nq[:, qs, hl * HD:(hl + 1) * HD],
                        av_reg(par, t)[:, 0:HD],
                        r[:, t:t + 1],
                    )
                if VARIANT == "debug" and u == 0:
                    avdump = onqpool.tile([P, 2, 512], f32, name="avdump_t")
                    nc.vector.tensor_copy(avdump[:], AVP[:, 0:2, :])
                    nc.sync.dma_start(dbg["avp0"][:], avdump[:].rearrange("p a b -> p (a b)"))
                    nc.sync.dma_start(dbg["onq0"][:], o_nq[:].rearrange("p a b -> p (a b)"))
                    nc.sync.dma_start(dbg["r0"][:], r[:])
                for qs in range(4):
                    _otrans(
                        onorm[:, p, ib * NB + qs * P:ib * NB + (qs + 1) * P],
                        o_nq[:, qs, :],
                    )

            # ---- Wo: one mo sub-block of output block n ----
            def wo_mm(n, mo, pool=None):
                if pool is None:
                    ps = opspool.tile([P, NB], f32, name="ops_t")
                else:
                    ps = pool.tile([P, 1024], f32, name="sc_t")[:, 0:NB]
                for k2 in range(2):
                    nc.tensor.matmul(
                        ps[:],
                        wo_sb[:, k2, mo * P:(mo + 1) * P],
                        onorm[:, k2, n * NB:(n + 1) * NB],
                        start=(k2 == 0),
                        stop=(k2 == 1),
                    )
                ot = otpool.tile([P, NB], f16, name="ot_t")
                nc.vector.tensor_copy(ot[:], ps[:])
                nc.sync.dma_start(out[mo * P:(mo + 1) * P, n * NB:(n + 1) * NB], ot[:])

            # ---- schedule ----
            def P_(wsb, xd, dst, bcol, m, n):
                return lambda: proj_qk_n(wsb, xd, dst, bcol, m, n)

            def V_(s):
                return lambda: v_block(s)

            def W_(n, mo):
                return lambda: wo_mm(n, mo)

            K = lambda m, n: P_(wk_sb, xk_q, kpt, 2, m, n)
            Q = lambda m, n: P_(wq_sb, xq_q, qpt, 0, m, n)

            # fills keyed by global j-step gj = u*16 + j  (0..127).
            # HARD RULE: the tile framework does NOT reorder; every producer
            # must be EMITTED before its consumer.  v_block(s) before
            # av_j(0, s) at gj=s+1; K(p, n) before scores_j((p,*), 4n);
            # Q(p, ib) before gj=unit*16.
            fills = {
                0: [V_(2)],
                1: [V_(3)],
                2: [V_(4)],
                3: [V_(5), K(0, 1)],
                4: [V_(6)],
                5: [V_(7)],
                6: [V_(8), K(0, 2)],
                7: [V_(9)],
                8: [V_(10)],
                9: [V_(11), K(0, 3)],
                10: [V_(12)],
                11: [V_(13)],
                12: [V_(14)],
                13: [V_(15)],
                14: [Q(0, 1)],
                28: [Q(0, 2)],
                34: [K(1, 0)],
                38: [K(1, 1)],
                42: [Q(0, 3)],
                46: [K(1, 2)],
                50: [K(1, 3)],
                54: [Q(1, 0)],
                58: [Q(1, 1)],
                62: [Q(1, 2)],
                66: [Q(1, 3)],
            }
            # Wo blocks n=0..2 spread into units 5..7.  norm(4+n) is emitted
            # inside window gj=(5+n)*16+1 AFTER that window's fills, so wo
            # fills must start at gj >= (5+n)*16+2.
            for n in range(3):
                base = (5 + n) * 16 + 2
                for mo in range(8):
                    fills.setdefault(base + mo * 13 // 8, []).append(W_(n, mo))

            # warmup: psum from the (idle) scpool ring to overlap drains
            av_zero(0)
            v_block(0, pool=scpool)
            v_block(1, pool=scpool)
            proj_qk_n(wk_sb, xk_q, kpt, 2, 0, 0, pool=scpool)
            proj_qk_n(wq_sb, xq_q, qpt, 0, 0, 0, pool=scpool)

            # AV runs with lag 2: av(gj-2)'s gate (exp(gj-2)) is already
            # satisfied when it reaches the in-order PE queue, so it never
            # head-of-line-blocks scores(gj+1) whose gate is exp(gj-1).
            for gj in range(128):
                u, j = divmod(gj, NJ)
                if j == 8 and u < 7:
                    av_zero(u + 1)  # zero next unit's parity regions early
                scores_j(u, j)
                exp_j(u, j)
                if gj >= 2:
                    u2, j2 = divmod(gj - 2, NJ)
                    av_j(u2, j2)
                    if j2 == NJ - 1:
                        norm(u2)
                for f in fills.get(gj, []):
                    f()

            # tail
            av_j(7, NJ - 2)
            av_j(7, NJ - 1)
            norm(7)
            for mo in range(8):
                wo_mm(3, mo, pool=scpool)
            if VARIANT == "debug":
                nc.sync.dma_start(dbg["qpt"][:], qpt[:].rearrange("p a b -> p (a b)"))
                nc.sync.dma_start(dbg["kpt"][:], kpt[:].rearrange("p a b -> p (a b)"))
                nc.sync.dma_start(dbg["vp"][:], vp[:].rearrange("p a b -> p (a b)"))
                nc.sync.dma_start(dbg["onorm"][:], onorm[:].rearrange("p a b -> p (a b)"))

    return nc


def _get_nc():
    if "nc" not in _nc_cache:
        _install_bir_fix()
        _nc_cache["nc"] = _build_nc()
    return _nc_cache["nc"]


# --------------------------------------------------------------------------
# Host wrapper
# --------------------------------------------------------------------------
def run(inputs, trace=False):
    from concourse.bass_utils import run_bass_kernel_spmd

    q = np.asarray(inputs["q"], np.float32)
    k = np.asarray(inputs["k"], np.float32)
    v = np.asarray(inputs["v"], np.float32)
    Wq = np.asarray(inputs["Wq"], np.float32)
    bq = np.asarray(inputs["bq"], np.float32)
    Wk = np.asarray(inputs["Wk"], np.float32)
    bk = np.asarray(inputs["bk"], np.float32)
    Wv = np.asarray(inputs["Wv"], np.float32)
    bv = np.asarray(inputs["bv"], np.float32)
    Wo = np.asarray(inputs["Wo"], np.float32)
    bo = np.asarray(inputs["bo"], np.float32)

    nc = _get_nc()

    xT = {}
    for b in range(2):
        xT[b] = (
            np.ascontiguousarray(q[b].T).astype(np.float16),
            np.ascontiguousarray(k[b].T).astype(np.float16),
            np.ascontiguousarray(v[b].T).astype(np.float16),
        )

    in_maps = []
    for c in range(8):
        b, g = divmod(c, 4)
        sl = slice(g * DL, (g + 1) * DL)
        bias = np.stack(
            [bq[sl][:P], bq[sl][P:], bk[sl][:P], bk[sl][P:]], axis=1
        ).astype(np.float32)
        in_maps.append({
            "xq": xT[b][0],
            "xk": xT[b][1],
            "xv": xT[b][2],
            "wqT": np.ascontiguousarray(Wq[sl, :].T).astype(np.float16),
            "wkT": np.ascontiguousarray(Wk[sl, :].T).astype(np.float16),
            "wvT": np.ascontiguousarray(Wv[sl, :].T).astype(np.float16),
            "woT": np.ascontiguousarray(Wo[:, sl].T).astype(np.float16),
            "bias": bias,
        })

    res = run_bass_kernel_spmd(
        nc, in_maps, core_ids=list(range(8)), trace=trace,
    )
    outs = [r["out"] for r in res.results]

    const = (Wo @ bv + bo).astype(np.float32)  # [1024]
    full = np.empty((2, S, H), np.float32)
    for b in range(2):
        acc = outs[4 * b].astype(np.float32).copy()
        for g in range(1, 4):
            acc += outs[4 * b + g]
        full[b] = acc.T + const
    return full, res


def kernel(**inputs):
    full, _ = run(inputs, trace=False)
    return full


# revision 29
# speedup vs baseline: 1.3419x; 1.0377x over previous
"""Multi-head attention (B=2, S=2048, H=1024, 16 heads) on 8 TRN2 NeuronCores.

Sharding: data-parallel over batch (2) x tensor-parallel over heads (16 -> 4
groups of 4 heads).  Core c = b*4 + g handles batch b, heads [4g, 4g+4).

v2 structure (fp16 storage / fp32 accumulate), x = q|k|v of its batch:
  QP_T[d, s] = (Wq_g x^T + bq_g)   d on partitions
  KP_T[d, s] = (Wk_g x^T + bk_g)
  VP[s, d]   = x Wv_g^T            natural layout + ones column per head
  per (pair p, i-block, j-tile):
     S_T[j, i] = K^T-contracted scores  (two heads share the PE via row tiles)
     E = exp(S_T/8)  on ACT, fused over two j-tiles  ([128, 2048] per instr)
     O[q, d+1] += E_j^T @ [V_j | 1]    transposed AV: q on psum partitions,
                                       ones column accumulates the denom L
  norm: r = 1/L (custom-DVE fast reciprocal), O_n = O * r (tensor_scalar),
        O_n^T via DMA-transpose (xbar) -> onorm[d, q]
  out_T[o, i] = Wo_g^T-contracted projection of onorm -> [1024, 2048] f32->f16
Host: out[b] = sum_g out_T(b,g)^T + (Wo @ bv + bo).
"""

import json

import numpy as np

S = 2048
H = 1024
DL = 256          # local projection dim = 4 heads * 64
P = 128
HD = 64
NK = H // P       # 8 k-tiles over hidden dim
NB = 512          # i-block (queries per attention unit column block)
NJ = S // P       # 16 j tiles of 128 keys
VW = 4 * (HD + 1)  # vp row width: 4 heads x (64 + ones col)
RS = 72           # AV psum region stride (32B aligned); region width 65

_nc_cache = {}
VARIANT = "full"  # debug bisect hook


# --------------------------------------------------------------------------
# BIR fix: this container's walrus supports only ONE sync wait (and update)
# per TPB instruction; Tile attaches several.  Split extras onto single-wait
# EventSemaphore instructions at the serialization boundary.
# --------------------------------------------------------------------------
_wsplit_counter = [0]


def _mk_evsem(engine, debug, wait=None, update=None):
    _wsplit_counter[0] += 1
    return {
        "debug": debug,
        "engine": engine,
        "ins": [],
        "outs": [],
        "name": f"wsplit-{_wsplit_counter[0]}",
        "opcode": "EventSemaphore",
        "sync_info": {
            "on_wait": [wait] if wait else [],
            "on_update": [update] if update else [],
        },
    }


def _split_bir_waits(bir):
    for f in bir.get("functions", []):
        for blk in f.get("blocks", []):
            out = []
            for inst in blk.get("instructions", []):
                si = inst.get("sync_info")
                waits = list(si.get("on_wait") or []) if si else []
                updates = list(si.get("on_update") or []) if si else []
                eng = inst.get("engine")
                dbg = inst.get("debug", 0)
                if len(waits) > 1:
                    for w in waits[:-1]:
                        out.append(_mk_evsem(eng, dbg, wait=w))
                    si["on_wait"] = [waits[-1]]
                out.append(inst)
                if len(updates) > 1:
                    si["on_update"] = [updates[0]]
                    for u in updates[1:]:
                        out.append(_mk_evsem(eng, dbg, update=u))
            blk["instructions"] = out
    return bir


def _install_bir_fix():
    import concourse.bass as bass

    if getattr(bass.Bass, "_wsplit_installed", False):
        return
    orig = bass.Bass.to_json_bytes

    def to_json_bytes(self, *a, **k):
        bir = json.loads(orig(self, *a, **k))
        return json.dumps(_split_bir_waits(bir)).encode()

    bass.Bass.to_json_bytes = to_json_bytes
    bass.Bass._wsplit_installed = True


# --------------------------------------------------------------------------
# Kernel builder
# --------------------------------------------------------------------------

def _build_nc():
    import concourse.bass as bass
    import concourse.mybir as mybir
    import concourse.tile as tile

    f16 = mybir.dt.float16
    f32 = mybir.dt.float32
    Exp = mybir.ActivationFunctionType.Exp

    nc = bass.Bass("TRN2")

    xq = nc.dram_tensor("xq", [H, S], f16, kind="ExternalInput")
    xk = nc.dram_tensor("xk", [H, S], f16, kind="ExternalInput")
    xv = nc.dram_tensor("xv", [H, S], f16, kind="ExternalInput")
    wqT = nc.dram_tensor("wqT", [H, DL], f16, kind="ExternalInput")
    wkT = nc.dram_tensor("wkT", [H, DL], f16, kind="ExternalInput")
    wvT = nc.dram_tensor("wvT", [H, DL], f16, kind="ExternalInput")
    woT = nc.dram_tensor("woT", [DL, H], f16, kind="ExternalInput")
    bias = nc.dram_tensor("bias", [P, 4], f32, kind="ExternalInput")  # bq0 bq1 bk0 bk1
    out = nc.dram_tensor("out", [H, S], f16, kind="ExternalOutput")
    dbg = {}
    if VARIANT == "debug":
        dbg["qpt"] = nc.dram_tensor("dbg_qpt", [P, 2 * S], f16, kind="ExternalOutput")
        dbg["kpt"] = nc.dram_tensor("dbg_kpt", [P, 2 * S], f16, kind="ExternalOutput")
        dbg["vp"] = nc.dram_tensor("dbg_vp", [P, NJ * VW], f16, kind="ExternalOutput")
        dbg["e0"] = nc.dram_tensor("dbg_e0", [P, 1024], f16, kind="ExternalOutput")
        dbg["onq0"] = nc.dram_tensor("dbg_onq0", [P, 4 * P], f16, kind="ExternalOutput")
        dbg["onorm"] = nc.dram_tensor("dbg_onorm", [P, 2 * S], f16, kind="ExternalOutput")
        dbg["r0"] = nc.dram_tensor("dbg_r0", [P, 8], f32, kind="ExternalOutput")
        dbg["avp0"] = nc.dram_tensor("dbg_avp0", [P, 2 * 512], f32, kind="ExternalOutput")

    UNITS = [(p, ib) for p in range(2) for ib in range(4)]  # p-major

    with tile.TileContext(nc) as tc:
        with (
            tc.tile_pool(name="persist", bufs=1) as persist,
            tc.tile_pool(name="xpool", bufs=96) as xpool,
            tc.tile_pool(name="epool", bufs=12) as epool,
            tc.tile_pool(name="onq", bufs=4) as onqpool,
            tc.tile_pool(name="rp", bufs=4) as rpool,
            tc.tile_pool(name="otp", bufs=3) as otpool,
            tc.tile_pool(name="scp", bufs=2, space="PSUM") as scpool,
            tc.tile_pool(name="avp", bufs=1, space="PSUM") as avpool,
            tc.tile_pool(name="opsp", bufs=1, space="PSUM") as opspool,
        ):
            # ---- persistent tiles ----
            wq_sb = persist.tile([P, NK, DL], f16, name="wq_sb")
            wk_sb = persist.tile([P, NK, DL], f16, name="wk_sb")
            wv_sb = persist.tile([P, NK, DL], f16, name="wv_sb")
            wo_sb = persist.tile([P, 2, H], f16, name="wo_sb")
            bias_sb = persist.tile([P, 4], f32, name="bias_sb")
            qpt = persist.tile([P, 2, S], f16, name="qpt")
            kpt = persist.tile([P, 2, S], f16, name="kpt")
            vp = persist.tile([P, NJ, VW], f16, name="vp")
            onorm = persist.tile([P, 2, S], f16, name="onorm")

            # psum: SC slots as pool-rotated tiles (2x2 banks; psum deps are
            # whole-tile, so slots must be separate tiles) + AV ping-pong
            # packed regions (3 banks)
            AVP = avpool.tile([P, 3, 512], f32, name="AVP")

            # ---- DMA loads, first-needed first ----
            nc.sync.dma_start(wv_sb[:], wvT.rearrange("(k p) d -> p k d", p=P))
            xv_q = {}
            xk_q = {}
            xq_q = {}

            def load_q(dst, src, k, n):
                t = xpool.tile([P, NB], f16, name="x_sb")
                nc.sync.dma_start(t[:], src[k * P:(k + 1) * P, n * NB:(n + 1) * NB])
                dst[(k, n)] = t

            for k in range(NK):
                load_q(xv_q, xv, k, 0)
            nc.sync.dma_start(wk_sb[:], wkT.rearrange("(k p) d -> p k d", p=P))
            for k in range(NK):
                load_q(xk_q, xk, k, 0)
            nc.sync.dma_start(wq_sb[:], wqT.rearrange("(k p) d -> p k d", p=P))
            for k in range(NK):
                load_q(xq_q, xq, k, 0)
            nc.sync.dma_start(bias_sb[:], bias[:])
            for k in range(NK):
                load_q(xv_q, xv, k, 1)
            for k in range(NK):
                load_q(xk_q, xk, k, 1)
            for k in range(NK):
                load_q(xv_q, xv, k, 2)
            for k in range(NK):
                load_q(xv_q, xv, k, 3)
            for k in range(NK):
                load_q(xk_q, xk, k, 2)
            for k in range(NK):
                load_q(xk_q, xk, k, 3)
            for n in range(1, 4):
                for k in range(NK):
                    load_q(xq_q, xq, k, n)
            nc.sync.dma_start(wo_sb[:], woT.rearrange("(k p) d -> p k d", p=P))

            # ones columns in vp (col 64 of each head block)
            for h in range(4):
                nc.gpsimd.memset(vp[:, :, h * (HD + 1) + HD:h * (HD + 1) + HD + 1], 1.0)

            # ---- AV psum region map: parity ping-pong over 3 banks ----
            def av_reg(par, t):
                if par == 0:
                    if t < 7:
                        return AVP[:, 0, RS * t:RS * t + 65]
                    return AVP[:, 1, 0:65]
                else:
                    if t < 6:
                        return AVP[:, 1, RS + RS * t:RS + RS * t + 65]
                    return AVP[:, 2, RS * (t - 6):RS * (t - 6) + 65]

            # ---- V projection for one s-tile ----
            def v_block(s, pool=None):
                n, c = divmod(s, 4)
                if pool is None:
                    ps = opspool.tile([P, NB], f32, name="ops_t")
                else:
                    ps = pool.tile([P, 1024], f32, name="sc_t")[:, 0:NB]
                for k in range(NK):
                    nc.tensor.matmul(
                        ps[:, :DL],
                        xv_q[(k, n)][:, c * P:(c + 1) * P],
                        wv_sb[:, k, :],
                        start=(k == 0),
                        stop=(k == NK - 1),
                    )
                for hg in range(4):
                    nc.vector.tensor_copy(
                        vp[:, s, hg * (HD + 1):hg * (HD + 1) + HD],
                        ps[:, hg * HD:(hg + 1) * HD],
                    )

            # ---- K/Q projection, one (m, n) block ----
            def proj_qk_n(wsb, xq_, dst, bcol, m, n, pool=None):
                if pool is None:
                    ps = opspool.tile([P, NB], f32, name="ops_t")
                else:
                    ps = pool.tile([P, 1024], f32, name="sc_t")[:, 0:NB]
                for k in range(NK):
                    nc.tensor.matmul(
                        ps[:],
                        wsb[:, k, m * P:(m + 1) * P],
                        xq_[(k, n)][:],
                        start=(k == 0),
                        stop=(k == NK - 1),
                    )
                nc.vector.tensor_scalar_add(
                    dst[:, m, n * NB:(n + 1) * NB], ps[:],
                    bias_sb[:, bcol + m:bcol + m + 1],
                )

            # ---- attention pieces ----
            e_tiles = {}
            sc_tiles = {}

            def scores_j(u, j):
                p, ib = UNITS[u]
                isl = slice(ib * NB, (ib + 1) * NB)
                jsl = slice(j * P, (j + 1) * P)
                sc = scpool.tile([P, 1024], f32, name="sc_t")
                nc.tensor.matmul(
                    sc[:, 0:NB],
                    kpt[0:HD, p, jsl], qpt[0:HD, p, isl],
                    start=True, stop=True,
                )
                nc.tensor.matmul(
                    sc[:, NB:2 * NB],
                    kpt[HD:P, p, jsl], qpt[HD:P, p, isl],
                    start=True, stop=True,
                )
                sc_tiles[(u, j)] = sc

            def exp_j(u, j):
                sc = sc_tiles.pop((u, j))
                e = epool.tile([P, 1024], f16, name="e_t")
                nc.scalar.activation(e[:], sc[:], Exp, scale=0.125)
                e_tiles[(u, j)] = e
                if VARIANT == "debug" and (u, j) == (0, 0):
                    nc.sync.dma_start(dbg["e0"][:], e[:])

            def av_j(u, j):
                p, ib = UNITS[u]
                par = u % 2
                e = e_tiles.pop((u, j))
                for t in range(8):
                    qs, hl = divmod(t, 2)
                    lhsT = e[:, hl * NB + qs * P:hl * NB + (qs + 1) * P]
                    hg = 2 * p + hl
                    # start=True would zero the WHOLE psum bank, wiping the
                    # sibling regions packed in it — accumulate over memset.
                    nc.tensor.matmul(
                        av_reg(par, t),
                        lhsT,
                        vp[:, j, hg * (HD + 1):hg * (HD + 1) + 65],
                        start=False,
                        stop=(j == NJ - 1),
                        skip_group_check=True,
                    )

            def av_zero(u):
                if u % 2 == 0:
                    nc.vector.memset(AVP[:, 0, 0:6 * RS + 65], 0.0)
                    nc.vector.memset(AVP[:, 1, 0:65], 0.0)
                else:
                    nc.vector.memset(AVP[:, 1, RS:6 * RS + 65], 0.0)
                    nc.vector.memset(AVP[:, 2, 0:RS + 65], 0.0)

            def flat(ap):
                return ap.rearrange("p t c -> p (t c)")

            def _recip(out_ap, in_ap):
                # custom-DVE (reciprocal_approx_fast) hits "ISA wrong length"
                # in this container's walrus; plain InstReciprocal on [128,<=8]
                # is cheap enough.
                nc.vector.reciprocal(out_ap, in_ap)

            def _otrans(dst, src):
                if VARIANT == "notrans":
                    nc.sync.dma_start(dst, src.rearrange("a b -> b a"))
                else:
                    nc.sync.dma_start_transpose(dst, src)

            def norm(u):
                p, ib = UNITS[u]
                par = u % 2
                r = rpool.tile([P, 8], f32, name="r_t")
                if par == 0:
                    l0 = AVP[:, 0, 0:7 * RS].rearrange(
                        "p (t c) -> p t c", c=RS)[:, :, 64:65]
                    _recip(r[:, 0:7], flat(l0))
                    _recip(r[:, 7:8], AVP[:, 1, 64:65])
                else:
                    l0 = AVP[:, 1, RS:7 * RS].rearrange(
                        "p (t c) -> p t c", c=RS)[:, :, 64:65]
                    _recip(r[:, 0:6], flat(l0))
                    l1 = AVP[:, 2, 0:2 * RS].rearrange(
                        "p (t c) -> p t c", c=RS)[:, :, 64:65]
                    _recip(r[:, 6:8], flat(l1))
                o_nq = onqpool.tile([P, 4, P], f16, name="onq_t")
                for t in range(8):
                    qs, hl = divmod(t, 2)
                    nc.vector.tensor_scalar_mul(
                        o_nq[:, qs, hl * HD:(hl + 1) * HD],
                        av_reg(par, t)[:, 0:HD],
                        r[:, t:t + 1],
                    )
                if VARIANT == "debug" and u == 0:
                    avdump = onqpool.tile([P, 2, 512], f32, name="avdump_t")
                    nc.vector.tensor_copy(avdump[:], AVP[:, 0:2, :])
                    nc.sync.dma_start(dbg["avp0"][:], avdump[:].rearrange("p a b -> p (a b)"))
                    nc.sync.dma_start(dbg["onq0"][:], o_nq[:].rearrange("p a b -> p (a b)"))
                    nc.sync.dma_start(dbg["r0"][:], r[:])
                for qs in range(4):
                    _otrans(
                        onorm[:, p, ib * NB + qs * P:ib * NB + (qs + 1) * P],
                        o_nq[:, qs, :],
                    )

            # ---- Wo: one mo sub-block of output block n ----
            def wo_mm(n, mo, pool=None):
                if pool is None:
                    ps = opspool.tile([P, NB], f32, name="ops_t")
                else:
                    ps = pool.tile([P, 1024], f32, name="sc_t")[:, 0:NB]
                for k2 in range(2):
                    nc.tensor.matmul(
                        ps[:],
                        wo_sb[:, k2, mo * P:(mo + 1) * P],
                        onorm[:, k2, n * NB:(n + 1) * NB],
                        start=(k2 == 0),
                        stop=(k2 == 1),
                    )
                ot = otpool.tile([P, NB], f16, name="ot_t")
                nc.vector.tensor_copy(ot[:], ps[:])
                nc.sync.dma_start(out[mo * P:(mo + 1) * P, n * NB:(n + 1) * NB], ot[:])

            # ---- schedule ----
            def P_(wsb, xd, dst, bcol, m, n):
                return lambda: proj_qk_n(wsb, xd, dst, bcol, m, n)

            def V_(s):
                return lambda: v_block(s)

            def W_(n, mo):
                return lambda: wo_mm(n, mo)

            K = lambda m, n: P_(wk_sb, xk_q, kpt, 2, m, n)
            Q = lambda m, n: P_(wq_sb, xq_q, qpt, 0, m, n)

            # fills keyed by global j-step gj = u*16 + j  (0..127).
            # HARD RULE: the tile framework does NOT reorder; every producer
            # must be EMITTED before its consumer.  v_block(s) before
            # av_j(0, s) at gj=s+1; K(p, n) before scores_j((p,*), 4n);
            # Q(p, ib) before gj=unit*16.
            fills = {
                0: [V_(2)],
                1: [V_(3)],
                2: [V_(4)],
                3: [V_(5), K(0, 1)],
                4: [V_(6)],
                5: [V_(7)],
                6: [V_(8)],
                7: [V_(9), K(0, 2)],
                8: [V_(10)],
                9: [V_(11)],
                10: [V_(12)],
                11: [V_(13), K(0, 3)],
                12: [V_(14)],
                13: [V_(15)],
                14: [Q(0, 1)],
                28: [Q(0, 2)],
                34: [K(1, 0)],
                38: [K(1, 1)],
                42: [Q(0, 3)],
                46: [K(1, 2)],
                50: [K(1, 3)],
                54: [Q(1, 0)],
                58: [Q(1, 1)],
                62: [Q(1, 2)],
                66: [Q(1, 3)],
            }
            # Wo blocks n=0..2 spread into units 5..7.  norm(4+n) is emitted
            # inside window gj=(5+n)*16+1 AFTER that window's fills, so wo
            # fills must start at gj >= (5+n)*16+2.
            for n in range(3):
                base = (5 + n) * 16 + 2
                for mo in range(8):
                    fills.setdefault(base + mo * 13 // 8, []).append(W_(n, mo))

            # warmup: psum from the (idle) scpool ring to overlap drains
            av_zero(0)
            v_block(0, pool=scpool)
            v_block(1, pool=scpool)
            proj_qk_n(wk_sb, xk_q, kpt, 2, 0, 0, pool=scpool)
            proj_qk_n(wq_sb, xq_q, qpt, 0, 0, 0, pool=scpool)

            # AV runs with lag 2: av(gj-2)'s gate (exp(gj-2)) is already
            # satisfied when it reaches the in-order PE queue, so it never
            # head-of-line-blocks scores(gj+1) whose gate is exp(gj-1).
            for gj in range(128):
                u, j = divmod(gj, NJ)
                if j == 8 and u < 7:
                    av_zero(u + 1)  # zero next unit's parity regions early
                scores_j(u, j)
                exp_j(u, j)
                if gj >= 2:
                    u2, j2 = divmod(gj - 2, NJ)
                    av_j(u2, j2)
                    if j2 == NJ - 1:
                        norm(u2)
                for f in fills.get(gj, []):
                    f()

            # tail
            av_j(7, NJ - 2)
            av_j(7, NJ - 1)
            norm(7)
            for mo in range(8):
                wo_mm(3, mo, pool=scpool)
            if VARIANT == "debug":
                nc.sync.dma_start(dbg["qpt"][:], qpt[:].rearrange("p a b -> p (a b)"))
                nc.sync.dma_start(dbg["kpt"][:], kpt[:].rearrange("p a b -> p (a b)"))
                nc.sync.dma_start(dbg["vp"][:], vp[:].rearrange("p a b -> p (a b)"))
                nc.sync.dma_start(dbg["onorm"][:], onorm[:].rearrange("p a b -> p (a b)"))

    return nc


def _get_nc():
    if "nc" not in _nc_cache:
        _install_bir_fix()
        _nc_cache["nc"] = _build_nc()
    return _nc_cache["nc"]


# --------------------------------------------------------------------------
# Host wrapper
# --------------------------------------------------------------------------
def run(inputs, trace=False):
    from concourse.bass_utils import run_bass_kernel_spmd

    q = np.asarray(inputs["q"], np.float32)
    k = np.asarray(inputs["k"], np.float32)
    v = np.asarray(inputs["v"], np.float32)
    Wq = np.asarray(inputs["Wq"], np.float32)
    bq = np.asarray(inputs["bq"], np.float32)
    Wk = np.asarray(inputs["Wk"], np.float32)
    bk = np.asarray(inputs["bk"], np.float32)
    Wv = np.asarray(inputs["Wv"], np.float32)
    bv = np.asarray(inputs["bv"], np.float32)
    Wo = np.asarray(inputs["Wo"], np.float32)
    bo = np.asarray(inputs["bo"], np.float32)

    nc = _get_nc()

    xT = {}
    for b in range(2):
        xT[b] = (
            np.ascontiguousarray(q[b].T).astype(np.float16),
            np.ascontiguousarray(k[b].T).astype(np.float16),
            np.ascontiguousarray(v[b].T).astype(np.float16),
        )

    in_maps = []
    for c in range(8):
        b, g = divmod(c, 4)
        sl = slice(g * DL, (g + 1) * DL)
        bias = np.stack(
            [bq[sl][:P], bq[sl][P:], bk[sl][:P], bk[sl][P:]], axis=1
        ).astype(np.float32)
        in_maps.append({
            "xq": xT[b][0],
            "xk": xT[b][1],
            "xv": xT[b][2],
            "wqT": np.ascontiguousarray(Wq[sl, :].T).astype(np.float16),
            "wkT": np.ascontiguousarray(Wk[sl, :].T).astype(np.float16),
            "wvT": np.ascontiguousarray(Wv[sl, :].T).astype(np.float16),
            "woT": np.ascontiguousarray(Wo[:, sl].T).astype(np.float16),
            "bias": bias,
        })

    res = run_bass_kernel_spmd(
        nc, in_maps, core_ids=list(range(8)), trace=trace,
    )
    outs = [r["out"] for r in res.results]

    const = (Wo @ bv + bo).astype(np.float32)  # [1024]
    full = np.empty((2, S, H), np.float32)
    for b in range(2):
        acc = outs[4 * b].astype(np.float32).copy()
        for g in range(1, 4):
            acc += outs[4 * b + g]
        full[b] = acc.T + const
    return full, res


def kernel(**inputs):
    full, _ = run(inputs, trace=False)
    return full
